# revision 34
# baseline (speedup 1.0000x reference)
"""Encoder layer (pre-norm attention + MLP) on 8 Trainium2 cores.

Sharding: core = (batch b in 0..3, half hf in 0..1). Each core receives the
full 2048-token sequence of batch b, transposed to [E, S] and rolled so the
core's own 1024 tokens are columns 0:1024 (attention and LN are invariant to
key order, so rolling keeps the program identical across cores). The core
computes K/V over the full sequence and everything else only for its own
tokens. No collectives; the host reassembles the 8 shards.

v2 redesign vs the first version:
- Everything lives in SBUF between stages (K/Q/V, h) — no DRAM round trips.
- LN1 + QKV projection fused into one chunk-pipelined stream so the tensor
  engine stays busy (HAM stays un-throttled at 2.4 GHz).
- Attention is software-pipelined (ctx matmuls lag scores by 2 steps) so the
  PE never waits on the scalar-engine exp.
- All weights bf16 (halves HBM traffic; matmul rate identical).
- x arrives as bf16 for the LN/projection path and f32 (own tokens only)
  for the residual path.
- fc2 evacuation fuses bias + residual: single output tensor.
"""

import numpy as np
import ml_dtypes
from contextlib import ExitStack

import concourse.bacc as bacc
import concourse.mybir as mybir
import concourse.tile as tile
from concourse.bass_utils import run_bass_kernel_spmd

F32 = mybir.dt.float32
F32R = mybir.dt.float32r
BF16 = mybir.dt.bfloat16
AF = mybir.ActivationFunctionType
OP = mybir.AluOpType

B, S, E, H, D, FF = 4, 2048, 1024, 16, 64, 4096
TOWN = 1024  # tokens owned per core
ET = E // 128  # 8
FT = FF // 128  # 32
NT = S // 128  # 16 token tiles (full seq)
NCORES = 8
EPS = 1e-6
CH = 512  # token chunk for the fused LN1+QKV pipeline
NCH = S // CH  # 4
OWN_CH = TOWN // CH  # 2 (chunks 0,1 are own tokens)


def _build():
    nc = bacc.Bacc()

    x_bf = nc.dram_tensor("x_bf", [E, S], BF16, kind="ExternalInput")
    x_own_d = nc.dram_tensor("x_own_d", [E, TOWN], F32R, kind="ExternalInput")
    wq_h = nc.dram_tensor("wq_h", [128, ET, ET, 128], BF16,
                          kind="ExternalInput")
    wk_h = nc.dram_tensor("wk_h", [128, ET, ET, 128], BF16,
                          kind="ExternalInput")
    wv_h = nc.dram_tensor("wv_h", [128, 2, ET, 512], BF16,
                          kind="ExternalInput")
    qb = nc.dram_tensor("qb", [128, ET], F32, kind="ExternalInput")
    kb = nc.dram_tensor("kb", [128, ET], F32, kind="ExternalInput")
    vb = nc.dram_tensor("vb", [E], F32R, kind="ExternalInput")
    wout_h = nc.dram_tensor("wout_h", [128, ET, ET, 128], BF16,
                            kind="ExternalInput")
    ob = nc.dram_tensor("ob", [128, ET], F32, kind="ExternalInput")
    wfc1_h = nc.dram_tensor("wfc1_h", [128, FT, ET, 128], BF16,
                            kind="ExternalInput")
    f1b = nc.dram_tensor("f1b", [128, FT], F32, kind="ExternalInput")
    wfc2_h = nc.dram_tensor("wfc2_h", [128, ET, FT, 128], BF16,
                            kind="ExternalInput")
    f2b = nc.dram_tensor("f2b", [128, ET], F32, kind="ExternalInput")

    out_d = nc.dram_tensor("out_d", [E, TOWN], F32, kind="ExternalOutput")

    inv_e = 1.0 / E
    unb = float(E) / (E - 1.0)  # unbiased-variance factor

    with tile.TileContext(nc) as tc, ExitStack() as ctx:
        consts = ctx.enter_context(tc.tile_pool(name="consts", bufs=1))

        # x2 = x + attention output; bf16, lives D..G. Opened early so the
        # pool stack stays LIFO. Same for z2 (LN2 output, lives E..F).
        p_x2 = ctx.enter_context(tc.tile_pool(name="p_x2", bufs=1))
        x2 = p_x2.tile([128, ET, TOWN], BF16)
        p_z2 = ctx.enter_context(tc.tile_pool(name="p_z2", bufs=1))
        z2 = p_z2.tile([128, ET, TOWN], BF16)

        # ============ Stage A+B: fused LN1 + QKV over chunk pipeline =======
        s_kqv = ExitStack()
        p_kqv = s_kqv.enter_context(tc.tile_pool(name="p_kqv", bufs=1))
        k_sb = p_kqv.tile([128, ET, S], BF16)
        q_sb = p_kqv.tile([128, ET, TOWN], BF16)
        # [part = t%128, t_tile, head, 64 v dims + 1 ones col]
        v_sb = p_kqv.tile([128, NT, H, 65], BF16)

        with tc.tile_pool(name="p_w", bufs=1) as p_w, \
             tc.tile_pool(name="p_wkq", bufs=3) as p_wkq, \
             tc.tile_pool(name="p_x", bufs=2) as p_x, \
             tc.tile_pool(name="p_xsq", bufs=8) as p_xsq, \
             tc.tile_pool(name="p_z", bufs=2) as p_z, \
             tc.tile_pool(name="p_st", bufs=1) as p_st, \
             tc.tile_pool(name="p_stps", bufs=2, space="PSUM") as p_stps, \
             tc.tile_pool(name="p_pp", bufs=4, space="PSUM") as p_pp:

            xre = x_bf.rearrange("(a p) s -> p a s", p=128)
            xc_t = [None] * NCH
            z1_t = [None] * NCH

            def load_chunk(c):
                # two half-DMAs so stats matmuls can start after the first
                # half lands (Tile tracks slice-level deps)
                xc = p_x.tile([128, ET, CH], BF16, tag="xc", name=f"xc{c}")
                csl = slice(c * CH, (c + 1) * CH)
                nc.sync.dma_start(out=xc[:, 0:ET // 2, :],
                                  in_=xre[:, 0:ET // 2, csl])
                nc.sync.dma_start(out=xc[:, ET // 2:ET, :],
                                  in_=xre[:, ET // 2:ET, csl])
                xc_t[c] = xc

            # x + V weights first on the DMA ring (the critical path);
            # constants and biases behind them.
            load_chunk(0)
            wv_sb = p_w.tile([128, 2, ET, 512], BF16)
            nc.sync.dma_start(out=wv_sb, in_=wv_h[:, :, :, :])
            load_chunk(1)

            ones_f32 = consts.tile([128, 256], F32)
            nc.vector.memset(ones_f32, 1.0)
            ones_bf = consts.tile([128, 128], BF16)
            nc.vector.tensor_copy(ones_bf, ones_f32[:, 0:128])
            ones_fr = consts.tile([128, 128], F32R)
            nc.vector.tensor_copy(ones_fr, ones_f32[:, 0:128])
            ones_col = consts.tile([128, 256], BF16)
            nc.vector.tensor_copy(ones_col, ones_f32)
            nc.vector.tensor_copy(
                v_sb[:, :, :, 64],
                ones_col[:, 0:NT * H].rearrange("p (a b) -> p a b", a=NT))
            qb_sb = consts.tile([128, ET], F32)
            kb_sb = consts.tile([128, ET], F32)
            ob_sb = consts.tile([128, ET], F32)
            f1b_sb = consts.tile([128, FT], F32)
            f2b_sb = consts.tile([128, ET], F32)
            nc.sync.dma_start(out=qb_sb, in_=qb[:, :])
            nc.sync.dma_start(out=kb_sb, in_=kb[:, :])
            nc.sync.dma_start(out=ob_sb, in_=ob[:, :])
            nc.sync.dma_start(out=f1b_sb, in_=f1b[:, :])
            nc.sync.dma_start(out=f2b_sb, in_=f2b[:, :])
            # v bias broadcast to all 128 token partitions (v is token-major)
            vb_row = consts.tile([1, E], F32R)
            nc.sync.dma_start(out=vb_row, in_=vb[None, :])
            vb_bc = consts.tile([128, E], F32)
            for c in range(2):
                ps = p_pp.tile([128, 512], F32, tag="pp", name=f"vbbc{c}")
                nc.tensor.matmul(ps, ones_fr[0:1, :],
                                 vb_row[:, c * 512:(c + 1) * 512],
                                 start=True, stop=True)
                nc.scalar.activation(vb_bc[:, c * 512:(c + 1) * 512], ps,
                                     AF.Copy)

            def stats(c):
                # LN1 stats for chunk c: ones-matmul sums (broadcast to all
                # 128 partitions for free), then mean/rstd in bf16.
                xc = xc_t[c]
                ps = p_stps.tile([128, 2, CH], F32, tag="st",
                                 name=f"st{c}")
                xsqs = []
                for a in range(ET):
                    xsq = p_xsq.tile([128, CH], BF16, tag="xsq")
                    nc.scalar.activation(xsq, xc[:, a, :], AF.Square)
                    xsqs.append(xsq)
                for a in range(ET):
                    nc.tensor.matmul(ps[:, 0, :], ones_bf, xc[:, a, :],
                                     start=(a == 0), stop=(a == ET - 1))
                for a in range(ET):
                    nc.tensor.matmul(ps[:, 1, :], ones_bf, xsqs[a],
                                     start=(a == 0), stop=(a == ET - 1))
                m_bf = p_st.tile([128, CH], BF16, tag="m", name=f"m{c}")
                nc.vector.tensor_scalar_mul(m_bf, ps[:, 0, :], inv_e)
                var = p_st.tile([128, CH], F32, tag="var")
                nc.vector.tensor_scalar_mul(var, ps[:, 1, :], 1.0 / (E - 1.0))
                msq = p_st.tile([128, CH], F32, tag="msq")
                nc.vector.tensor_tensor(msq, m_bf, m_bf, OP.mult)
                nc.vector.scalar_tensor_tensor(var, msq, -unb, var,
                                               OP.mult, OP.add)
                std = p_st.tile([128, CH], F32, tag="std")
                nc.scalar.activation(std, var, AF.Sqrt)
                rstd = p_st.tile([128, CH], F32, tag="rstd")
                nc.vector.reciprocal_approx_fast(rstd, std)
                rstd_bf = p_st.tile([128, CH], BF16, tag="rstdb",
                                    name=f"r{c}")
                nc.vector.tensor_copy(rstd_bf, rstd)
                # normalize
                z1 = p_z.tile([128, ET, CH], BF16, tag="z1", name=f"z1{c}")
                for a in range(ET):
                    nc.vector.tensor_tensor(z1[:, a, :], xc[:, a, :], m_bf,
                                            OP.subtract)
                    nc.vector.tensor_tensor(z1[:, a, :], z1[:, a, :], rstd_bf,
                                            OP.mult)
                z1_t[c] = z1

            def proj(c):
                z1 = z1_t[c]
                csl = slice(c * CH, (c + 1) * CH)
                # K projection (feature-major output); weights streamed
                for ot in range(ET):
                    wt = p_wkq.tile([128, ET, 128], BF16, tag="w",
                                    name=f"wk{c}_{ot}")
                    nc.sync.dma_start(out=wt, in_=wk_h[:, ot])
                    ps = p_pp.tile([128, CH], F32, tag="pp",
                                   name=f"k{c}_{ot}")
                    for a in range(ET):
                        nc.tensor.matmul(ps, wt[:, a, :], z1[:, a, :],
                                         start=(a == 0), stop=(a == ET - 1))
                    nc.scalar.activation(k_sb[:, ot, csl], ps, AF.Identity,
                                         bias=kb_sb[:, ot:ot + 1])
                # V projection (token-major output, straight into v_sb)
                for tt in range(CH // 128):
                    t_abs = c * (CH // 128) + tt
                    tsl = slice(tt * 128, (tt + 1) * 128)
                    for half in range(2):
                        ps = p_pp.tile([128, 512], F32, tag="pp",
                                       name=f"v{c}_{tt}_{half}")
                        for a in range(ET):
                            nc.tensor.matmul(ps, z1[:, a, tsl],
                                             wv_sb[:, half, a, :],
                                             start=(a == 0),
                                             stop=(a == ET - 1))
                        nc.vector.tensor_tensor(
                            v_sb[:, t_abs, half * 8:(half + 1) * 8, 0:64],
                            ps.rearrange("p (h w) -> p h w", w=64),
                            vb_bc[:, half * 512:(half + 1) * 512].rearrange(
                                "p (h w) -> p h w", w=64),
                            OP.add)
                # Q projection (own chunks only); weights streamed
                if c < OWN_CH:
                    for ot in range(ET):
                        wt = p_wkq.tile([128, ET, 128], BF16, tag="w",
                                        name=f"wq{c}_{ot}")
                        nc.sync.dma_start(out=wt, in_=wq_h[:, ot])
                        ps = p_pp.tile([128, CH], F32, tag="pp",
                                       name=f"q{c}_{ot}")
                        for a in range(ET):
                            nc.tensor.matmul(ps, wt[:, a, :], z1[:, a, :],
                                             start=(a == 0),
                                             stop=(a == ET - 1))
                        nc.scalar.activation(q_sb[:, ot, csl], ps,
                                             AF.Identity,
                                             bias=qb_sb[:, ot:ot + 1])

            # chunk pipeline: stats one chunk ahead of projections
            stats(0)
            load_chunk(2)
            stats(1)
            load_chunk(3)
            proj(0)
            stats(2)
            proj(1)
            stats(3)
            proj(2)
            proj(3)

        # ============ Stage C: attention ===================================
        # ctxn + out-proj weights + residual x, all live C..D.
        s_cd = ExitStack()
        p_cd = s_cd.enter_context(tc.tile_pool(name="p_cd", bufs=1))
        ctxn = p_cd.tile([128, ET, TOWN], BF16)
        x_own = p_cd.tile([128, ET, TOWN], F32R)
        nc.sync.dma_start(out=x_own,
                          in_=x_own_d.rearrange("(a p) s -> p a s", p=128))
        wout_sb = p_cd.tile([128, ET, ET, 128], BF16)
        nc.sync.dma_start(out=wout_sb, in_=wout_h[:, :, :, :])

        with tc.tile_pool(name="p_pr", bufs=4) as p_pr, \
             tc.tile_pool(name="p_cm", bufs=2) as p_cm, \
             tc.tile_pool(name="p_pss", bufs=3, space="PSUM") as p_pss, \
             tc.tile_pool(name="p_psx", bufs=2, space="PSUM") as p_psx:

            def block(qc, h):
                qsl = slice(qc * 512, (qc + 1) * 512)
                lo = (h % 2) * 64
                hsl = slice(lo, lo + 64)
                ot = h // 2
                ctx_ps = p_psx.tile([65, 512], F32, tag="ctx",
                                    name=f"c{qc}_{h}")
                # software pipeline: scores run 2 steps ahead of ctx so
                # the PE never waits on the scalar-engine exp.
                pr = [None] * (NT // 2)

                def scores(k2):
                    sp = p_pss.tile([128, 2, 512], F32, tag="s",
                                    name=f"s{qc}_{h}_{k2}")
                    for j in range(2):
                        kt = 2 * k2 + j
                        nc.tensor.matmul(
                            sp[:, j, :],
                            k_sb[hsl, ot, kt * 128:(kt + 1) * 128],
                            q_sb[hsl, ot, qsl], start=True, stop=True)
                    p = p_pr.tile([128, 2, 512], BF16, tag="pr")
                    nc.scalar.activation(p, sp, AF.Exp, scale=0.125)
                    pr[k2] = p

                def ctxmm(k2):
                    p = pr[k2]
                    for j in range(2):
                        kt = 2 * k2 + j
                        nc.tensor.matmul(ctx_ps, v_sb[:, kt, h, :],
                                         p[:, j, :],
                                         start=(kt == 0),
                                         stop=(kt == NT - 1))

                scores(0)
                scores(1)
                scores(2)
                for k2 in range(3, NT // 2):
                    scores(k2)
                    ctxmm(k2 - 3)
                ctxmm(NT // 2 - 3)
                ctxmm(NT // 2 - 2)
                ctxmm(NT // 2 - 1)

                # softmax denominator: recip on DVE, partition-broadcast
                # on the (otherwise idle) GpSimd engine — no PSUM needed.
                # (reciprocal_approx_fast misreads partition-offset
                # inputs, so stage the denominator at partition 0 first)
                den = p_cm.tile([1, 512], F32, tag="den")
                nc.vector.tensor_copy(den, ctx_ps[64:65, :])
                rec = p_cm.tile([1, 512], F32, tag="rec")
                nc.vector.reciprocal_approx_fast(rec, den)
                rb = p_cm.tile([64, 512], F32, tag="rbs")
                nc.gpsimd.partition_broadcast(rb, rec)
                nc.vector.tensor_tensor(ctxn[hsl, ot, qsl],
                                        ctx_ps[0:64, :], rb, OP.mult)

            for qc in range(2):
                for h in range(H):
                    block(qc, h)

        def ln2_pass(qc, st_pool, sq_pool, ps_pool):
            qsl = slice(qc * 512, (qc + 1) * 512)
            ps = ps_pool.tile([128, 2, 512], F32, tag="e", name=f"e{qc}")
            xsqs = []
            for a in range(ET):
                xsq = sq_pool.tile([128, 512], BF16, tag="xsq")
                nc.vector.tensor_tensor(xsq, x2[:, a, qsl],
                                        x2[:, a, qsl], OP.mult)
                xsqs.append(xsq)
            for a in range(ET):
                nc.tensor.matmul(ps[:, 0, :], ones_bf, x2[:, a, qsl],
                                 start=(a == 0), stop=(a == ET - 1))
            for a in range(ET):
                nc.tensor.matmul(ps[:, 1, :], ones_bf, xsqs[a],
                                 start=(a == 0), stop=(a == ET - 1))
            m2 = st_pool.tile([128, 512], BF16, tag="m2")
            nc.vector.tensor_scalar_mul(m2, ps[:, 0, :], inv_e)
            var = st_pool.tile([128, 512], F32, tag="var2")
            nc.vector.tensor_scalar_mul(var, ps[:, 1, :], 1.0 / (E - 1.0))
            msq = st_pool.tile([128, 512], F32, tag="msq2")
            nc.vector.tensor_tensor(msq, m2, m2, OP.mult)
            nc.vector.scalar_tensor_tensor(var, msq, -unb, var,
                                           OP.mult, OP.add)
            std = st_pool.tile([128, 512], F32, tag="std2")
            nc.scalar.activation(std, var, AF.Sqrt)
            rstd = st_pool.tile([128, 512], F32, tag="rstd2")
            nc.vector.reciprocal_approx_fast(rstd, std)
            rstd_bf = st_pool.tile([128, 512], BF16, tag="rstd2b")
            nc.vector.tensor_copy(rstd_bf, rstd)
            for a in range(ET):
                nc.vector.tensor_tensor(z2[:, a, qsl], x2[:, a, qsl],
                                        m2, OP.subtract)
                nc.vector.tensor_tensor(z2[:, a, qsl], z2[:, a, qsl],
                                        rstd_bf, OP.mult)

        # ======= Stage D: out-proj + residual, LN2(qc0) interleaved ========
        # E(qc0)'s DVE finalize/normalize runs under D(qc1)'s matmuls.
        with tc.tile_pool(name="p_dps", bufs=2, space="PSUM") as p_dps, \
             tc.tile_pool(name="p_est", bufs=1) as p_est, \
             tc.tile_pool(name="p_esq", bufs=4) as p_esq, \
             tc.tile_pool(name="p_eps0", bufs=1, space="PSUM") as p_eps0:

            def dpass(qc):
                qsl = slice(qc * 512, (qc + 1) * 512)
                for ot in range(ET):
                    ps = p_dps.tile([128, 512], F32, tag="d",
                                    name=f"d{qc}_{ot}")
                    for a in range(ET):
                        nc.tensor.matmul(ps, wout_sb[:, ot, a, :],
                                         ctxn[:, a, qsl],
                                         start=(a == 0), stop=(a == ET - 1))
                    nc.vector.scalar_tensor_tensor(
                        x2[:, ot, qsl], ps, ob_sb[:, ot:ot + 1],
                        x_own[:, ot, qsl], OP.add, OP.add)

            dpass(0)
            ln2_pass(0, p_est, p_esq, p_eps0)
            dpass(1)
        s_cd.close()   # ctxn/x_own/wout dead
        s_kqv.close()  # k/q/v dead

        # ============ Stage E: LN2(qc1); Stage F: fc1+gelu -> h_sb =========
        p_h = ctx.enter_context(tc.tile_pool(name="p_h", bufs=1))
        h_sb = p_h.tile([128, FT, TOWN], BF16)

        FG = 4  # fc1 weight tiles per DMA group
        with tc.tile_pool(name="p_dst", bufs=2) as p_dst, \
             tc.tile_pool(name="p_dsq", bufs=8) as p_dsq, \
             tc.tile_pool(name="p_f1w", bufs=3) as p_f1w, \
             tc.tile_pool(name="p_eps", bufs=2, space="PSUM") as p_eps, \
             tc.tile_pool(name="p_fps", bufs=4, space="PSUM") as p_fps:
            wgs = [None] * (FT // FG)

            def load_wg(g):
                wg = p_f1w.tile([128, FG, ET, 128], BF16, tag="w",
                                name=f"wf1_{g}")
                nc.sync.dma_start(out=wg, in_=wfc1_h[:, g * FG:(g + 1) * FG])
                wgs[g] = wg

            load_wg(0)  # prefetch during LN2 so fc1 never stalls
            load_wg(1)
            ln2_pass(1, p_dst, p_dsq, p_eps)

            for g in range(FT // FG):
                if g + 2 < FT // FG:
                    load_wg(g + 2)
                wg = wgs[g]
                for fl in range(FG):
                    ft = g * FG + fl
                    for qc in range(2):
                        qsl = slice(qc * 512, (qc + 1) * 512)
                        ps = p_fps.tile([128, 512], F32, tag="f",
                                        name=f"f{ft}_{qc}")
                        for a in range(ET):
                            nc.tensor.matmul(ps, wg[:, fl, a, :],
                                             z2[:, a, qsl],
                                             start=(a == 0),
                                             stop=(a == ET - 1))
                        nc.scalar.activation(h_sb[:, ft, qsl], ps, AF.Gelu,
                                             bias=f1b_sb[:, ft:ft + 1])

        # ============ Stage G: fc2 + bias + residual -> out ================
        with tc.tile_pool(name="p_f2w", bufs=2) as p_f2w, \
             tc.tile_pool(name="p_ge", bufs=4) as p_ge, \
             tc.tile_pool(name="p_gps", bufs=4, space="PSUM") as p_gps:
            for ot in range(ET):
                w2 = p_f2w.tile([128, FT, 128], BF16, tag="w",
                                name=f"wf2_{ot}")
                nc.sync.dma_start(out=w2, in_=wfc2_h[:, ot])
                for qc in range(2):
                    qsl = slice(qc * 512, (qc + 1) * 512)
                    ps = p_gps.tile([128, 512], F32, tag="g",
                                    name=f"g{ot}_{qc}")
                    for f in range(FT):
                        nc.tensor.matmul(ps, w2[:, f, :], h_sb[:, f, qsl],
                                         start=(f == 0), stop=(f == FT - 1))
                    ev = p_ge.tile([128, 512], F32, tag="ev")
                    nc.scalar.activation(ev, ps, AF.Identity,
                                         bias=f2b_sb[:, ot:ot + 1])
                    outt = p_ge.tile([128, 512], F32, tag="out")
                    nc.vector.tensor_tensor(outt, ev, x2[:, ot, qsl], OP.add)
                    nc.sync.dma_start(
                        out=out_d[ot * 128:(ot + 1) * 128, qsl], in_=outt)

    nc.finalize()
    return nc


_NC_CACHE = {}


def _get_nc():
    if "k" not in _NC_CACHE:
        _NC_CACHE["k"] = _build()
    return _NC_CACHE["k"]


def _tile_w(w_t, n_out_tiles, inner):
    # [E_in, O] (in-feature rows) -> [128, O//inner_t, E_in//128, inner] with
    # partition (e_in % 128) leading so every DMA is contiguous per partition.
    e_in, o = w_t.shape
    arr = w_t.reshape(e_in // 128, 128, n_out_tiles, o // n_out_tiles)
    return np.ascontiguousarray(arr.transpose(1, 2, 0, 3)
                                ).astype(ml_dtypes.bfloat16)


def _prepare_in_maps(inputs):
    f = np.float32
    x = np.asarray(inputs["x"], f)
    w_qkv = np.asarray(inputs["w_qkv"], np.float64)
    ln1_w = np.asarray(inputs["ln1_w"], np.float64)
    ln1_b = np.asarray(inputs["ln1_b"], np.float64)
    ln2_w = np.asarray(inputs["ln2_w"], np.float64)
    ln2_b = np.asarray(inputs["ln2_b"], np.float64)
    w_fc1 = np.asarray(inputs["w_fc1"], np.float64)

    wqkv_s = (w_qkv * ln1_w[None, :])  # fold LN1 gamma
    qkv_bias = ln1_b @ np.asarray(inputs["w_qkv"], np.float64).T  # [3E]
    wqkv_t = np.ascontiguousarray(wqkv_s.T, f)  # [E, 3E]
    wq_hh = _tile_w(wqkv_t[:, 0:E], ET, 128)
    wk_hh = _tile_w(wqkv_t[:, E:2 * E], ET, 128)
    wv_hh = _tile_w(wqkv_t[:, 2 * E:3 * E], 2, 512)
    col = lambda v: np.ascontiguousarray(
        np.asarray(v, f).reshape(-1, 128).T)  # [o] -> [128, o//128]
    qb_a = col(qkv_bias[0:E])
    kb_a = col(qkv_bias[E:2 * E])
    vb_a = np.ascontiguousarray(qkv_bias[2 * E:3 * E], f)

    wout_hh = _tile_w(np.ascontiguousarray(np.asarray(inputs["w_out"], f).T),
                      ET, 128)
    ob_a = col(inputs["b_out"])

    wfc1_s = (w_fc1 * ln2_w[None, :])
    f1b_flat = np.asarray(inputs["b_fc1"], np.float64) + ln2_b @ w_fc1.T
    f1b_a = col(f1b_flat)
    wfc1_hh = _tile_w(np.ascontiguousarray(wfc1_s.T, f), FT, 128)
    wfc2_hh = _tile_w(np.ascontiguousarray(np.asarray(inputs["w_fc2"], f).T),
                      ET, 128)
    f2b_a = col(inputs["b_fc2"])

    shared = dict(wq_h=wq_hh, wk_h=wk_hh, wv_h=wv_hh, qb=qb_a, kb=kb_a,
                  vb=vb_a, wout_h=wout_hh, ob=ob_a, wfc1_h=wfc1_hh,
                  f1b=f1b_a, wfc2_h=wfc2_hh, f2b=f2b_a)
    in_maps = []
    for core in range(NCORES):
        b, hf = divmod(core, 2)
        xs = np.roll(x[b], -hf * TOWN, axis=0)  # own tokens first; [S, E]
        x_bfc = np.ascontiguousarray(xs.T.astype(ml_dtypes.bfloat16))
        x_own = np.ascontiguousarray(xs[0:TOWN].T)  # [E, TOWN] f32
        in_maps.append(dict(x_bf=x_bfc, x_own_d=x_own, **shared))
    return in_maps


def _assemble(inputs, results):
    f = np.float32
    out = np.empty((B, S, E), f)
    for core in range(NCORES):
        b, hf = divmod(core, 2)
        out[b, hf * TOWN:(hf + 1) * TOWN, :] = results[core]["out_d"].T
    return out


def run(inputs, **spmd_kwargs):
    nc = _get_nc()
    in_maps = _prepare_in_maps(inputs)
    res = run_bass_kernel_spmd(nc, in_maps, core_ids=list(range(NCORES)),
                               **spmd_kwargs)
    return _assemble(inputs, res.results), res


def kernel(**inputs):
    out, _ = run(inputs)
    return out


# revision 35
# speedup vs baseline: 1.2712x; 1.2712x over previous
"""Encoder layer (pre-norm attention + MLP) on 8 Trainium2 cores.

Sharding: core = (batch b in 0..3, half hf in 0..1). Each core receives the
full 2048-token sequence of batch b, transposed to [E, S] and rolled so the
core's own 1024 tokens are columns 0:1024 (attention and LN are invariant to
key order, so rolling keeps the program identical across cores). The core
computes K/V over the full sequence and everything else only for its own
tokens. No collectives; the host reassembles the 8 shards.

v2 redesign vs the first version:
- Everything lives in SBUF between stages (K/Q/V, h) — no DRAM round trips.
- LN1 + QKV projection fused into one chunk-pipelined stream so the tensor
  engine stays busy (HAM stays un-throttled at 2.4 GHz).
- Attention is software-pipelined (ctx matmuls lag scores by 2 steps) so the
  PE never waits on the scalar-engine exp.
- All weights bf16 (halves HBM traffic; matmul rate identical).
- x arrives as bf16 for the LN/projection path and f32 (own tokens only)
  for the residual path.
- fc2 evacuation fuses bias + residual: single output tensor.
"""

import numpy as np
import ml_dtypes
from contextlib import ExitStack

import concourse.bacc as bacc
import concourse.mybir as mybir
import concourse.tile as tile
from concourse.bass_utils import run_bass_kernel_spmd

F32 = mybir.dt.float32
F32R = mybir.dt.float32r
BF16 = mybir.dt.bfloat16
AF = mybir.ActivationFunctionType
OP = mybir.AluOpType

B, S, E, H, D, FF = 4, 2048, 1024, 16, 64, 4096
TOWN = 1024  # tokens owned per core
ET = E // 128  # 8
FT = FF // 128  # 32
NT = S // 128  # 16 token tiles (full seq)
NCORES = 8
EPS = 1e-6
CH = 512  # token chunk for the fused LN1+QKV pipeline
NCH = S // CH  # 4
OWN_CH = TOWN // CH  # 2 (chunks 0,1 are own tokens)


def _build():
    nc = bacc.Bacc()

    x_bf = nc.dram_tensor("x_bf", [E, S], BF16, kind="ExternalInput")
    x_own_d = nc.dram_tensor("x_own_d", [E, TOWN], F32R, kind="ExternalInput")
    wq_h = nc.dram_tensor("wq_h", [128, ET, ET, 128], BF16,
                          kind="ExternalInput")
    wk_h = nc.dram_tensor("wk_h", [128, ET, ET, 128], BF16,
                          kind="ExternalInput")
    wv_h = nc.dram_tensor("wv_h", [128, 2, ET, 512], BF16,
                          kind="ExternalInput")
    qb = nc.dram_tensor("qb", [128, ET], F32, kind="ExternalInput")
    kb = nc.dram_tensor("kb", [128, ET], F32, kind="ExternalInput")
    vb = nc.dram_tensor("vb", [E], F32R, kind="ExternalInput")
    wout_h = nc.dram_tensor("wout_h", [128, ET, ET, 128], BF16,
                            kind="ExternalInput")
    ob = nc.dram_tensor("ob", [128, ET], F32, kind="ExternalInput")
    wfc1_h = nc.dram_tensor("wfc1_h", [128, FT, ET, 128], BF16,
                            kind="ExternalInput")
    f1b = nc.dram_tensor("f1b", [128, FT], F32, kind="ExternalInput")
    wfc2_h = nc.dram_tensor("wfc2_h", [128, ET, FT, 128], BF16,
                            kind="ExternalInput")
    f2b = nc.dram_tensor("f2b", [128, ET], F32, kind="ExternalInput")

    out_d = nc.dram_tensor("out_d", [E, TOWN], F32, kind="ExternalOutput")

    inv_e = 1.0 / E
    unb = float(E) / (E - 1.0)  # unbiased-variance factor

    with tile.TileContext(nc) as tc, ExitStack() as ctx:
        consts = ctx.enter_context(tc.tile_pool(name="consts", bufs=1))

        # x2 = x + attention output; bf16, lives D..G. Opened early so the
        # pool stack stays LIFO.
        p_x2 = ctx.enter_context(tc.tile_pool(name="p_x2", bufs=1))
        x2 = p_x2.tile([128, ET, TOWN], BF16)

        # ============ Stage A+B: fused LN1 + QKV over chunk pipeline =======
        s_kqv = ExitStack()
        p_kqv = s_kqv.enter_context(tc.tile_pool(name="p_kqv", bufs=1))
        k_sb = p_kqv.tile([128, ET, S], BF16)
        q_sb = p_kqv.tile([128, ET, TOWN], BF16)
        # [part = t%128, t_tile, head, 64 v dims + 1 ones col]
        v_sb = p_kqv.tile([128, NT, H, 65], BF16)

        with tc.tile_pool(name="p_w", bufs=1) as p_w, \
             tc.tile_pool(name="p_wkq", bufs=3) as p_wkq, \
             tc.tile_pool(name="p_x", bufs=2) as p_x, \
             tc.tile_pool(name="p_xsq", bufs=8) as p_xsq, \
             tc.tile_pool(name="p_z", bufs=2) as p_z, \
             tc.tile_pool(name="p_st", bufs=1) as p_st, \
             tc.tile_pool(name="p_stps", bufs=2, space="PSUM") as p_stps, \
             tc.tile_pool(name="p_pp", bufs=4, space="PSUM") as p_pp:

            xre = x_bf.rearrange("(a p) s -> p a s", p=128)
            xc_t = [None] * NCH
            z1_t = [None] * NCH

            def load_chunk(c):
                xc = p_x.tile([128, ET, CH], BF16, tag="xc", name=f"xc{c}")
                nc.sync.dma_start(out=xc, in_=xre[:, :, c * CH:(c + 1) * CH])
                xc_t[c] = xc

            # x + V weights first on the DMA ring (the critical path);
            # constants and biases behind them.
            load_chunk(0)
            wv_sb = p_w.tile([128, 2, ET, 512], BF16)
            nc.sync.dma_start(out=wv_sb, in_=wv_h[:, :, :, :])
            load_chunk(1)

            ones_f32 = consts.tile([128, 256], F32)
            nc.vector.memset(ones_f32, 1.0)
            ones_bf = consts.tile([128, 128], BF16)
            nc.vector.tensor_copy(ones_bf, ones_f32[:, 0:128])
            ones_fr = consts.tile([128, 128], F32R)
            nc.vector.tensor_copy(ones_fr, ones_f32[:, 0:128])
            ones_col = consts.tile([128, 256], BF16)
            nc.vector.tensor_copy(ones_col, ones_f32)
            nc.vector.tensor_copy(
                v_sb[:, :, :, 64],
                ones_col[:, 0:NT * H].rearrange("p (a b) -> p a b", a=NT))
            qb_sb = consts.tile([128, ET], F32)
            kb_sb = consts.tile([128, ET], F32)
            ob_sb = consts.tile([128, ET], F32)
            f1b_sb = consts.tile([128, FT], F32)
            f2b_sb = consts.tile([128, ET], F32)
            nc.sync.dma_start(out=qb_sb, in_=qb[:, :])
            nc.sync.dma_start(out=kb_sb, in_=kb[:, :])
            nc.sync.dma_start(out=ob_sb, in_=ob[:, :])
            nc.sync.dma_start(out=f1b_sb, in_=f1b[:, :])
            nc.sync.dma_start(out=f2b_sb, in_=f2b[:, :])
            # v bias broadcast to all 128 token partitions (v is token-major)
            vb_row = consts.tile([1, E], F32R)
            nc.sync.dma_start(out=vb_row, in_=vb[None, :])
            vb_bc = consts.tile([128, E], F32)
            for c in range(2):
                ps = p_pp.tile([128, 512], F32, tag="pp", name=f"vbbc{c}")
                nc.tensor.matmul(ps, ones_fr[0:1, :],
                                 vb_row[:, c * 512:(c + 1) * 512],
                                 start=True, stop=True)
                nc.scalar.activation(vb_bc[:, c * 512:(c + 1) * 512], ps,
                                     AF.Copy)

            def stats(c):
                # LN1 stats for chunk c: ones-matmul sums (broadcast to all
                # 128 partitions for free), then mean/rstd in bf16.
                xc = xc_t[c]
                ps = p_stps.tile([128, 2, CH], F32, tag="st",
                                 name=f"st{c}")
                xsqs = []
                for a in range(ET):
                    xsq = p_xsq.tile([128, CH], BF16, tag="xsq")
                    nc.scalar.activation(xsq, xc[:, a, :], AF.Square)
                    xsqs.append(xsq)
                for a in range(ET):
                    nc.tensor.matmul(ps[:, 0, :], ones_bf, xc[:, a, :],
                                     start=(a == 0), stop=(a == ET - 1))
                for a in range(ET):
                    nc.tensor.matmul(ps[:, 1, :], ones_bf, xsqs[a],
                                     start=(a == 0), stop=(a == ET - 1))
                m_bf = p_st.tile([128, CH], BF16, tag="m", name=f"m{c}")
                nc.vector.tensor_scalar_mul(m_bf, ps[:, 0, :], inv_e)
                var = p_st.tile([128, CH], F32, tag="var")
                nc.vector.tensor_scalar_mul(var, ps[:, 1, :], 1.0 / (E - 1.0))
                msq = p_st.tile([128, CH], F32, tag="msq")
                nc.vector.tensor_tensor(msq, m_bf, m_bf, OP.mult)
                nc.vector.scalar_tensor_tensor(var, msq, -unb, var,
                                               OP.mult, OP.add)
                std = p_st.tile([128, CH], F32, tag="std")
                nc.scalar.activation(std, var, AF.Sqrt)
                rstd = p_st.tile([128, CH], F32, tag="rstd")
                nc.vector.reciprocal_approx_fast(rstd, std)
                rstd_bf = p_st.tile([128, CH], BF16, tag="rstdb",
                                    name=f"r{c}")
                nc.vector.tensor_copy(rstd_bf, rstd)
                # normalize
                z1 = p_z.tile([128, ET, CH], BF16, tag="z1", name=f"z1{c}")
                for a in range(ET):
                    nc.vector.tensor_tensor(z1[:, a, :], xc[:, a, :], m_bf,
                                            OP.subtract)
                    nc.vector.tensor_tensor(z1[:, a, :], z1[:, a, :], rstd_bf,
                                            OP.mult)
                z1_t[c] = z1

            def proj(c):
                z1 = z1_t[c]
                csl = slice(c * CH, (c + 1) * CH)
                # K projection (feature-major output); weights streamed
                for ot in range(ET):
                    wt = p_wkq.tile([128, ET, 128], BF16, tag="w",
                                    name=f"wk{c}_{ot}")
                    nc.sync.dma_start(out=wt, in_=wk_h[:, ot])
                    ps = p_pp.tile([128, CH], F32, tag="pp",
                                   name=f"k{c}_{ot}")
                    for a in range(ET):
                        nc.tensor.matmul(ps, wt[:, a, :], z1[:, a, :],
                                         start=(a == 0), stop=(a == ET - 1))
                    nc.scalar.activation(k_sb[:, ot, csl], ps, AF.Identity,
                                         bias=kb_sb[:, ot:ot + 1])
                # V projection (token-major output, straight into v_sb)
                for tt in range(CH // 128):
                    t_abs = c * (CH // 128) + tt
                    tsl = slice(tt * 128, (tt + 1) * 128)
                    for half in range(2):
                        ps = p_pp.tile([128, 512], F32, tag="pp",
                                       name=f"v{c}_{tt}_{half}")
                        for a in range(ET):
                            nc.tensor.matmul(ps, z1[:, a, tsl],
                                             wv_sb[:, half, a, :],
                                             start=(a == 0),
                                             stop=(a == ET - 1))
                        nc.vector.tensor_tensor(
                            v_sb[:, t_abs, half * 8:(half + 1) * 8, 0:64],
                            ps.rearrange("p (h w) -> p h w", w=64),
                            vb_bc[:, half * 512:(half + 1) * 512].rearrange(
                                "p (h w) -> p h w", w=64),
                            OP.add)
                # Q projection (own chunks only); weights streamed
                if c < OWN_CH:
                    for ot in range(ET):
                        wt = p_wkq.tile([128, ET, 128], BF16, tag="w",
                                        name=f"wq{c}_{ot}")
                        nc.sync.dma_start(out=wt, in_=wq_h[:, ot])
                        ps = p_pp.tile([128, CH], F32, tag="pp",
                                       name=f"q{c}_{ot}")
                        for a in range(ET):
                            nc.tensor.matmul(ps, wt[:, a, :], z1[:, a, :],
                                             start=(a == 0),
                                             stop=(a == ET - 1))
                        nc.scalar.activation(q_sb[:, ot, csl], ps,
                                             AF.Identity,
                                             bias=qb_sb[:, ot:ot + 1])

            # chunk pipeline: stats one chunk ahead of projections
            stats(0)
            load_chunk(2)
            stats(1)
            load_chunk(3)
            proj(0)
            stats(2)
            proj(1)
            stats(3)
            proj(2)
            proj(3)

        # ============ Stage C: attention ===================================
        # ctxn + out-proj weights + residual x, all live C..D.
        s_cd = ExitStack()
        p_cd = s_cd.enter_context(tc.tile_pool(name="p_cd", bufs=1))
        ctxn = p_cd.tile([128, ET, TOWN], BF16)
        x_own = p_cd.tile([128, ET, TOWN], F32R)
        nc.sync.dma_start(out=x_own,
                          in_=x_own_d.rearrange("(a p) s -> p a s", p=128))
        wout_sb = p_cd.tile([128, ET, ET, 128], BF16)
        nc.sync.dma_start(out=wout_sb, in_=wout_h[:, :, :, :])

        with tc.tile_pool(name="p_pr", bufs=4) as p_pr, \
             tc.tile_pool(name="p_cm", bufs=4) as p_cm, \
             tc.tile_pool(name="p_pss", bufs=3, space="PSUM") as p_pss, \
             tc.tile_pool(name="p_psx", bufs=2, space="PSUM") as p_psx:

            def block(qc, h):
                qsl = slice(qc * 512, (qc + 1) * 512)
                lo = (h % 2) * 64
                hsl = slice(lo, lo + 64)
                ot = h // 2
                ctx_ps = p_psx.tile([65, 512], F32, tag="ctx",
                                    name=f"c{qc}_{h}")
                # software pipeline: scores run 2 steps ahead of ctx so
                # the PE never waits on the scalar-engine exp.
                pr = [None] * (NT // 2)

                def scores(k2):
                    sp = p_pss.tile([128, 2, 512], F32, tag="s",
                                    name=f"s{qc}_{h}_{k2}")
                    for j in range(2):
                        kt = 2 * k2 + j
                        nc.tensor.matmul(
                            sp[:, j, :],
                            k_sb[hsl, ot, kt * 128:(kt + 1) * 128],
                            q_sb[hsl, ot, qsl], start=True, stop=True)
                    p = p_pr.tile([128, 2, 512], BF16, tag="pr")
                    nc.scalar.activation(p, sp, AF.Exp, scale=0.125)
                    pr[k2] = p

                def ctxmm(k2):
                    p = pr[k2]
                    for j in range(2):
                        kt = 2 * k2 + j
                        nc.tensor.matmul(ctx_ps, v_sb[:, kt, h, :],
                                         p[:, j, :],
                                         start=(kt == 0),
                                         stop=(kt == NT - 1))

                scores(0)
                scores(1)
                scores(2)
                for k2 in range(3, NT // 2):
                    scores(k2)
                    ctxmm(k2 - 3)
                ctxmm(NT // 2 - 3)
                ctxmm(NT // 2 - 2)
                ctxmm(NT // 2 - 1)

                # softmax denominator: recip on DVE, partition-broadcast
                # on the (otherwise idle) GpSimd engine — no PSUM needed.
                # (reciprocal_approx_fast misreads partition-offset
                # inputs, so stage the denominator at partition 0 first)
                den = p_cm.tile([1, 512], F32, tag="den")
                nc.vector.tensor_copy(den, ctx_ps[64:65, :])
                rec = p_cm.tile([1, 512], F32, tag="rec")
                nc.vector.reciprocal_approx_fast(rec, den)
                rb = p_cm.tile([64, 512], F32, tag="rbs")
                nc.gpsimd.partition_broadcast(rb, rec)
                nc.vector.tensor_tensor(ctxn[hsl, ot, qsl],
                                        ctx_ps[0:64, :], rb, OP.mult)

            for qc in range(2):
                for h in range(H):
                    block(qc, h)

        # ============ Stage D: out-proj + residual (evac on DVE) ===========
        with tc.tile_pool(name="p_dps", bufs=4, space="PSUM") as p_dps:
            for qc in range(2):
                qsl = slice(qc * 512, (qc + 1) * 512)
                for ot in range(ET):
                    ps = p_dps.tile([128, 512], F32, tag="d",
                                    name=f"d{qc}_{ot}")
                    for a in range(ET):
                        nc.tensor.matmul(ps, wout_sb[:, ot, a, :],
                                         ctxn[:, a, qsl],
                                         start=(a == 0), stop=(a == ET - 1))
                    nc.vector.scalar_tensor_tensor(
                        x2[:, ot, qsl], ps, ob_sb[:, ot:ot + 1],
                        x_own[:, ot, qsl], OP.add, OP.add)
        s_cd.close()   # ctxn/x_own/wout dead
        s_kqv.close()  # k/q/v dead

        # ============ Stage E: LN2 -> z2; Stage F: fc1+gelu -> h_sb ========
        p_z2 = ctx.enter_context(tc.tile_pool(name="p_z2", bufs=1))
        z2 = p_z2.tile([128, ET, TOWN], BF16)
        p_h = ctx.enter_context(tc.tile_pool(name="p_h", bufs=1))
        h_sb = p_h.tile([128, FT, TOWN], BF16)

        FG = 4  # fc1 weight tiles per DMA group
        with tc.tile_pool(name="p_dst", bufs=2) as p_dst, \
             tc.tile_pool(name="p_dsq", bufs=8) as p_dsq, \
             tc.tile_pool(name="p_f1w", bufs=3) as p_f1w, \
             tc.tile_pool(name="p_eps", bufs=2, space="PSUM") as p_eps, \
             tc.tile_pool(name="p_fps", bufs=4, space="PSUM") as p_fps:
            wgs = [None] * (FT // FG)

            def load_wg(g):
                wg = p_f1w.tile([128, FG, ET, 128], BF16, tag="w",
                                name=f"wf1_{g}")
                nc.sync.dma_start(out=wg, in_=wfc1_h[:, g * FG:(g + 1) * FG])
                wgs[g] = wg

            load_wg(0)  # prefetch during LN2 so fc1 never stalls
            load_wg(1)
            for qc in range(2):
                qsl = slice(qc * 512, (qc + 1) * 512)
                ps = p_eps.tile([128, 2, 512], F32, tag="e", name=f"e{qc}")
                xsqs = []
                for a in range(ET):
                    xsq = p_dsq.tile([128, 512], BF16, tag="xsq")
                    nc.vector.tensor_tensor(xsq, x2[:, a, qsl],
                                            x2[:, a, qsl], OP.mult)
                    xsqs.append(xsq)
                for a in range(ET):
                    nc.tensor.matmul(ps[:, 0, :], ones_bf, x2[:, a, qsl],
                                     start=(a == 0), stop=(a == ET - 1))
                for a in range(ET):
                    nc.tensor.matmul(ps[:, 1, :], ones_bf, xsqs[a],
                                     start=(a == 0), stop=(a == ET - 1))
                m2 = p_dst.tile([128, 512], BF16, tag="m2")
                nc.vector.tensor_scalar_mul(m2, ps[:, 0, :], inv_e)
                var = p_dst.tile([128, 512], F32, tag="var2")
                nc.vector.tensor_scalar_mul(var, ps[:, 1, :], 1.0 / (E - 1.0))
                msq = p_dst.tile([128, 512], F32, tag="msq2")
                nc.vector.tensor_tensor(msq, m2, m2, OP.mult)
                nc.vector.scalar_tensor_tensor(var, msq, -unb, var,
                                               OP.mult, OP.add)
                std = p_dst.tile([128, 512], F32, tag="std2")
                nc.scalar.activation(std, var, AF.Sqrt)
                rstd = p_dst.tile([128, 512], F32, tag="rstd2")
                nc.vector.reciprocal_approx_fast(rstd, std)
                rstd_bf = p_dst.tile([128, 512], BF16, tag="rstd2b")
                nc.vector.tensor_copy(rstd_bf, rstd)
                for a in range(ET):
                    nc.vector.tensor_tensor(z2[:, a, qsl], x2[:, a, qsl],
                                            m2, OP.subtract)
                    nc.vector.tensor_tensor(z2[:, a, qsl], z2[:, a, qsl],
                                            rstd_bf, OP.mult)

            for g in range(FT // FG):
                if g + 2 < FT // FG:
                    load_wg(g + 2)
                wg = wgs[g]
                for fl in range(FG):
                    ft = g * FG + fl
                    for qc in range(2):
                        qsl = slice(qc * 512, (qc + 1) * 512)
                        ps = p_fps.tile([128, 512], F32, tag="f",
                                        name=f"f{ft}_{qc}")
                        for a in range(ET):
                            nc.tensor.matmul(ps, wg[:, fl, a, :],
                                             z2[:, a, qsl],
                                             start=(a == 0),
                                             stop=(a == ET - 1))
                        nc.scalar.activation(h_sb[:, ft, qsl], ps, AF.Gelu,
                                             bias=f1b_sb[:, ft:ft + 1])

        # ============ Stage G: fc2 + bias + residual -> out ================
        with tc.tile_pool(name="p_f2w", bufs=2) as p_f2w, \
             tc.tile_pool(name="p_ge", bufs=4) as p_ge, \
             tc.tile_pool(name="p_gps", bufs=4, space="PSUM") as p_gps:
            for ot in range(ET):
                w2 = p_f2w.tile([128, FT, 128], BF16, tag="w",
                                name=f"wf2_{ot}")
                nc.sync.dma_start(out=w2, in_=wfc2_h[:, ot])
                for qc in range(2):
                    qsl = slice(qc * 512, (qc + 1) * 512)
                    ps = p_gps.tile([128, 512], F32, tag="g",
                                    name=f"g{ot}_{qc}")
                    for f in range(FT):
                        nc.tensor.matmul(ps, w2[:, f, :], h_sb[:, f, qsl],
                                         start=(f == 0), stop=(f == FT - 1))
                    ev = p_ge.tile([128, 512], F32, tag="ev")
                    nc.scalar.activation(ev, ps, AF.Identity,
                                         bias=f2b_sb[:, ot:ot + 1])
                    outt = p_ge.tile([128, 512], F32, tag="out")
                    nc.vector.tensor_tensor(outt, ev, x2[:, ot, qsl], OP.add)
                    nc.sync.dma_start(
                        out=out_d[ot * 128:(ot + 1) * 128, qsl], in_=outt)

    nc.finalize()
    return nc


_NC_CACHE = {}


def _get_nc():
    if "k" not in _NC_CACHE:
        _NC_CACHE["k"] = _build()
    return _NC_CACHE["k"]


def _tile_w(w_t, n_out_tiles, inner):
    # [E_in, O] (in-feature rows) -> [128, O//inner_t, E_in//128, inner] with
    # partition (e_in % 128) leading so every DMA is contiguous per partition.
    e_in, o = w_t.shape
    arr = w_t.reshape(e_in // 128, 128, n_out_tiles, o // n_out_tiles)
    return np.ascontiguousarray(arr.transpose(1, 2, 0, 3)
                                ).astype(ml_dtypes.bfloat16)


def _prepare_in_maps(inputs):
    f = np.float32
    x = np.asarray(inputs["x"], f)
    w_qkv = np.asarray(inputs["w_qkv"], np.float64)
    ln1_w = np.asarray(inputs["ln1_w"], np.float64)
    ln1_b = np.asarray(inputs["ln1_b"], np.float64)
    ln2_w = np.asarray(inputs["ln2_w"], np.float64)
    ln2_b = np.asarray(inputs["ln2_b"], np.float64)
    w_fc1 = np.asarray(inputs["w_fc1"], np.float64)

    wqkv_s = (w_qkv * ln1_w[None, :])  # fold LN1 gamma
    qkv_bias = ln1_b @ np.asarray(inputs["w_qkv"], np.float64).T  # [3E]
    wqkv_t = np.ascontiguousarray(wqkv_s.T, f)  # [E, 3E]
    wq_hh = _tile_w(wqkv_t[:, 0:E], ET, 128)
    wk_hh = _tile_w(wqkv_t[:, E:2 * E], ET, 128)
    wv_hh = _tile_w(wqkv_t[:, 2 * E:3 * E], 2, 512)
    col = lambda v: np.ascontiguousarray(
        np.asarray(v, f).reshape(-1, 128).T)  # [o] -> [128, o//128]
    qb_a = col(qkv_bias[0:E])
    kb_a = col(qkv_bias[E:2 * E])
    vb_a = np.ascontiguousarray(qkv_bias[2 * E:3 * E], f)

    wout_hh = _tile_w(np.ascontiguousarray(np.asarray(inputs["w_out"], f).T),
                      ET, 128)
    ob_a = col(inputs["b_out"])

    wfc1_s = (w_fc1 * ln2_w[None, :])
    f1b_flat = np.asarray(inputs["b_fc1"], np.float64) + ln2_b @ w_fc1.T
    f1b_a = col(f1b_flat)
    wfc1_hh = _tile_w(np.ascontiguousarray(wfc1_s.T, f), FT, 128)
    wfc2_hh = _tile_w(np.ascontiguousarray(np.asarray(inputs["w_fc2"], f).T),
                      ET, 128)
    f2b_a = col(inputs["b_fc2"])

    shared = dict(wq_h=wq_hh, wk_h=wk_hh, wv_h=wv_hh, qb=qb_a, kb=kb_a,
                  vb=vb_a, wout_h=wout_hh, ob=ob_a, wfc1_h=wfc1_hh,
                  f1b=f1b_a, wfc2_h=wfc2_hh, f2b=f2b_a)
    in_maps = []
    for core in range(NCORES):
        b, hf = divmod(core, 2)
        xs = np.roll(x[b], -hf * TOWN, axis=0)  # own tokens first; [S, E]
        x_bfc = np.ascontiguousarray(xs.T.astype(ml_dtypes.bfloat16))
        x_own = np.ascontiguousarray(xs[0:TOWN].T)  # [E, TOWN] f32
        in_maps.append(dict(x_bf=x_bfc, x_own_d=x_own, **shared))
    return in_maps


def _assemble(inputs, results):
    f = np.float32
    out = np.empty((B, S, E), f)
    for core in range(NCORES):
        b, hf = divmod(core, 2)
        out[b, hf * TOWN:(hf + 1) * TOWN, :] = results[core]["out_d"].T
    return out


def run(inputs, **spmd_kwargs):
    nc = _get_nc()
    in_maps = _prepare_in_maps(inputs)
    res = run_bass_kernel_spmd(nc, in_maps, core_ids=list(range(NCORES)),
                               **spmd_kwargs)
    return _assemble(inputs, res.results), res


def kernel(**inputs):
    out, _ = run(inputs)
    return out


# revision 36
# speedup vs baseline: 1.2747x; 1.0028x over previous
"""Encoder layer (pre-norm attention + MLP) on 8 Trainium2 cores.

Sharding: core = (batch b in 0..3, half hf in 0..1). Each core receives the
full 2048-token sequence of batch b, transposed to [E, S] and rolled so the
core's own 1024 tokens are columns 0:1024 (attention and LN are invariant to
key order, so rolling keeps the program identical across cores). The core
computes K/V over the full sequence and everything else only for its own
tokens. No collectives; the host reassembles the 8 shards.

v2 redesign vs the first version:
- Everything lives in SBUF between stages (K/Q/V, h) — no DRAM round trips.
- LN1 + QKV projection fused into one chunk-pipelined stream so the tensor
  engine stays busy (HAM stays un-throttled at 2.4 GHz).
- Attention is software-pipelined (ctx matmuls lag scores by 2 steps) so the
  PE never waits on the scalar-engine exp.
- All weights bf16 (halves HBM traffic; matmul rate identical).
- x arrives as bf16 for the LN/projection path and f32 (own tokens only)
  for the residual path.
- fc2 evacuation fuses bias + residual: single output tensor.
"""

import numpy as np
import ml_dtypes
from contextlib import ExitStack

import concourse.bacc as bacc
import concourse.mybir as mybir
import concourse.tile as tile
from concourse.bass_utils import run_bass_kernel_spmd

F32 = mybir.dt.float32
F32R = mybir.dt.float32r
BF16 = mybir.dt.bfloat16
AF = mybir.ActivationFunctionType
OP = mybir.AluOpType

B, S, E, H, D, FF = 4, 2048, 1024, 16, 64, 4096
TOWN = 1024  # tokens owned per core
ET = E // 128  # 8
FT = FF // 128  # 32
NT = S // 128  # 16 token tiles (full seq)
NCORES = 8
EPS = 1e-6
CH = 512  # token chunk for the fused LN1+QKV pipeline
NCH = S // CH  # 4
OWN_CH = TOWN // CH  # 2 (chunks 0,1 are own tokens)


def _build():
    nc = bacc.Bacc()

    x_bf = nc.dram_tensor("x_bf", [E, S], BF16, kind="ExternalInput")
    x_own_d = nc.dram_tensor("x_own_d", [E, TOWN], F32R, kind="ExternalInput")
    wq_h = nc.dram_tensor("wq_h", [128, ET, ET, 128], BF16,
                          kind="ExternalInput")
    wk_h = nc.dram_tensor("wk_h", [128, ET, ET, 128], BF16,
                          kind="ExternalInput")
    wv_h = nc.dram_tensor("wv_h", [128, 2, ET, 512], BF16,
                          kind="ExternalInput")
    qb = nc.dram_tensor("qb", [128, ET], F32, kind="ExternalInput")
    kb = nc.dram_tensor("kb", [128, ET], F32, kind="ExternalInput")
    vb = nc.dram_tensor("vb", [E], F32R, kind="ExternalInput")
    wout_h = nc.dram_tensor("wout_h", [128, ET, ET, 128], BF16,
                            kind="ExternalInput")
    ob = nc.dram_tensor("ob", [128, ET], F32, kind="ExternalInput")
    wfc1_h = nc.dram_tensor("wfc1_h", [128, FT, ET, 128], BF16,
                            kind="ExternalInput")
    f1b = nc.dram_tensor("f1b", [128, FT], F32, kind="ExternalInput")
    wfc2_h = nc.dram_tensor("wfc2_h", [128, ET, FT, 128], BF16,
                            kind="ExternalInput")
    f2b = nc.dram_tensor("f2b", [128, ET], F32, kind="ExternalInput")

    out_d = nc.dram_tensor("out_d", [E, TOWN], F32, kind="ExternalOutput")

    inv_e = 1.0 / E
    unb = float(E) / (E - 1.0)  # unbiased-variance factor

    with tile.TileContext(nc) as tc, ExitStack() as ctx:
        consts = ctx.enter_context(tc.tile_pool(name="consts", bufs=1))

        # x2 = x + attention output; bf16, lives D..G. Opened early so the
        # pool stack stays LIFO.
        p_x2 = ctx.enter_context(tc.tile_pool(name="p_x2", bufs=1))
        x2 = p_x2.tile([128, ET, TOWN], BF16)

        # ============ Stage A+B: fused LN1 + QKV over chunk pipeline =======
        s_kqv = ExitStack()
        p_kqv = s_kqv.enter_context(tc.tile_pool(name="p_kqv", bufs=1))
        k_sb = p_kqv.tile([128, ET, S], BF16)
        q_sb = p_kqv.tile([128, ET, TOWN], BF16)
        # [part = t%128, t_tile, head, 64 v dims + 1 ones col]
        v_sb = p_kqv.tile([128, NT, H, 65], BF16)

        with tc.tile_pool(name="p_w", bufs=1) as p_w, \
             tc.tile_pool(name="p_wkq", bufs=3) as p_wkq, \
             tc.tile_pool(name="p_x", bufs=2) as p_x, \
             tc.tile_pool(name="p_xsq", bufs=8) as p_xsq, \
             tc.tile_pool(name="p_z", bufs=2) as p_z, \
             tc.tile_pool(name="p_st", bufs=1) as p_st, \
             tc.tile_pool(name="p_stps", bufs=2, space="PSUM") as p_stps, \
             tc.tile_pool(name="p_pp", bufs=4, space="PSUM") as p_pp:

            xre = x_bf.rearrange("(a p) s -> p a s", p=128)
            xc_t = [None] * NCH
            z1_t = [None] * NCH

            def load_chunk(c):
                xc = p_x.tile([128, ET, CH], BF16, tag="xc", name=f"xc{c}")
                nc.sync.dma_start(out=xc, in_=xre[:, :, c * CH:(c + 1) * CH])
                xc_t[c] = xc

            # x + V weights first on the DMA ring (the critical path);
            # constants and biases behind them.
            load_chunk(0)
            wv_sb = p_w.tile([128, 2, ET, 512], BF16)
            nc.sync.dma_start(out=wv_sb, in_=wv_h[:, :, :, :])
            load_chunk(1)

            ones_f32 = consts.tile([128, 256], F32)
            nc.vector.memset(ones_f32, 1.0)
            ones_bf = consts.tile([128, 128], BF16)
            nc.vector.tensor_copy(ones_bf, ones_f32[:, 0:128])
            ones_fr = consts.tile([128, 128], F32R)
            nc.vector.tensor_copy(ones_fr, ones_f32[:, 0:128])
            ones_col = consts.tile([128, 256], BF16)
            nc.vector.tensor_copy(ones_col, ones_f32)
            nc.vector.tensor_copy(
                v_sb[:, :, :, 64],
                ones_col[:, 0:NT * H].rearrange("p (a b) -> p a b", a=NT))
            qb_sb = consts.tile([128, ET], F32)
            kb_sb = consts.tile([128, ET], F32)
            ob_sb = consts.tile([128, ET], F32)
            f1b_sb = consts.tile([128, FT], F32)
            f2b_sb = consts.tile([128, ET], F32)
            nc.sync.dma_start(out=qb_sb, in_=qb[:, :])
            nc.sync.dma_start(out=kb_sb, in_=kb[:, :])
            nc.sync.dma_start(out=ob_sb, in_=ob[:, :])
            nc.sync.dma_start(out=f1b_sb, in_=f1b[:, :])
            nc.sync.dma_start(out=f2b_sb, in_=f2b[:, :])
            # v bias broadcast to all 128 token partitions (v is token-major)
            vb_row = consts.tile([1, E], F32R)
            nc.sync.dma_start(out=vb_row, in_=vb[None, :])
            vb_bc = consts.tile([128, E], F32)
            for c in range(2):
                ps = p_pp.tile([128, 512], F32, tag="pp", name=f"vbbc{c}")
                nc.tensor.matmul(ps, ones_fr[0:1, :],
                                 vb_row[:, c * 512:(c + 1) * 512],
                                 start=True, stop=True)
                nc.scalar.activation(vb_bc[:, c * 512:(c + 1) * 512], ps,
                                     AF.Copy)

            def stats(c):
                # LN1 stats for chunk c: ones-matmul sums (broadcast to all
                # 128 partitions for free), then mean/rstd in bf16.
                xc = xc_t[c]
                ps = p_stps.tile([128, 2, CH], F32, tag="st",
                                 name=f"st{c}")
                xsqs = []
                for a in range(ET):
                    xsq = p_xsq.tile([128, CH], BF16, tag="xsq")
                    nc.scalar.activation(xsq, xc[:, a, :], AF.Square)
                    xsqs.append(xsq)
                for a in range(ET):
                    nc.tensor.matmul(ps[:, 0, :], ones_bf, xc[:, a, :],
                                     start=(a == 0), stop=(a == ET - 1))
                for a in range(ET):
                    nc.tensor.matmul(ps[:, 1, :], ones_bf, xsqs[a],
                                     start=(a == 0), stop=(a == ET - 1))
                m_bf = p_st.tile([128, CH], BF16, tag="m", name=f"m{c}")
                nc.vector.tensor_scalar_mul(m_bf, ps[:, 0, :], inv_e)
                var = p_st.tile([128, CH], F32, tag="var")
                nc.vector.tensor_scalar_mul(var, ps[:, 1, :], 1.0 / (E - 1.0))
                msq = p_st.tile([128, CH], F32, tag="msq")
                nc.vector.tensor_tensor(msq, m_bf, m_bf, OP.mult)
                nc.vector.scalar_tensor_tensor(var, msq, -unb, var,
                                               OP.mult, OP.add)
                std = p_st.tile([128, CH], F32, tag="std")
                nc.scalar.activation(std, var, AF.Sqrt)
                rstd = p_st.tile([128, CH], F32, tag="rstd")
                nc.vector.reciprocal_approx_fast(rstd, std)
                rstd_bf = p_st.tile([128, CH], BF16, tag="rstdb",
                                    name=f"r{c}")
                nc.vector.tensor_copy(rstd_bf, rstd)
                # normalize
                z1 = p_z.tile([128, ET, CH], BF16, tag="z1", name=f"z1{c}")
                for a in range(ET):
                    nc.vector.tensor_tensor(z1[:, a, :], xc[:, a, :], m_bf,
                                            OP.subtract)
                    nc.vector.tensor_tensor(z1[:, a, :], z1[:, a, :], rstd_bf,
                                            OP.mult)
                z1_t[c] = z1

            def proj(c):
                z1 = z1_t[c]
                csl = slice(c * CH, (c + 1) * CH)
                # K projection (feature-major output); weights streamed
                for ot in range(ET):
                    wt = p_wkq.tile([128, ET, 128], BF16, tag="w",
                                    name=f"wk{c}_{ot}")
                    nc.sync.dma_start(out=wt, in_=wk_h[:, ot])
                    ps = p_pp.tile([128, CH], F32, tag="pp",
                                   name=f"k{c}_{ot}")
                    for a in range(ET):
                        nc.tensor.matmul(ps, wt[:, a, :], z1[:, a, :],
                                         start=(a == 0), stop=(a == ET - 1))
                    nc.scalar.activation(k_sb[:, ot, csl], ps, AF.Identity,
                                         bias=kb_sb[:, ot:ot + 1])
                # V projection (token-major output, straight into v_sb)
                for tt in range(CH // 128):
                    t_abs = c * (CH // 128) + tt
                    tsl = slice(tt * 128, (tt + 1) * 128)
                    for half in range(2):
                        ps = p_pp.tile([128, 512], F32, tag="pp",
                                       name=f"v{c}_{tt}_{half}")
                        for a in range(ET):
                            nc.tensor.matmul(ps, z1[:, a, tsl],
                                             wv_sb[:, half, a, :],
                                             start=(a == 0),
                                             stop=(a == ET - 1))
                        nc.vector.tensor_tensor(
                            v_sb[:, t_abs, half * 8:(half + 1) * 8, 0:64],
                            ps.rearrange("p (h w) -> p h w", w=64),
                            vb_bc[:, half * 512:(half + 1) * 512].rearrange(
                                "p (h w) -> p h w", w=64),
                            OP.add)
                # Q projection (own chunks only); weights streamed
                if c < OWN_CH:
                    for ot in range(ET):
                        wt = p_wkq.tile([128, ET, 128], BF16, tag="w",
                                        name=f"wq{c}_{ot}")
                        nc.sync.dma_start(out=wt, in_=wq_h[:, ot])
                        ps = p_pp.tile([128, CH], F32, tag="pp",
                                       name=f"q{c}_{ot}")
                        for a in range(ET):
                            nc.tensor.matmul(ps, wt[:, a, :], z1[:, a, :],
                                             start=(a == 0),
                                             stop=(a == ET - 1))
                        nc.scalar.activation(q_sb[:, ot, csl], ps,
                                             AF.Identity,
                                             bias=qb_sb[:, ot:ot + 1])

            # chunk pipeline: stats one chunk ahead of projections
            # x-chunk loads are emitted AFTER the preceding projection so
            # proj's streamed K/Q weight DMAs aren't queued behind 1MB
            # x transfers on the FIFO DMA ring (each xc still lands well
            # before the stats pass that consumes it).
            stats(0)
            stats(1)
            proj(0)
            load_chunk(2)
            stats(2)
            proj(1)
            load_chunk(3)
            stats(3)
            proj(2)
            proj(3)

        # ============ Stage C: attention ===================================
        # ctxn + out-proj weights + residual x, all live C..D.
        s_cd = ExitStack()
        p_cd = s_cd.enter_context(tc.tile_pool(name="p_cd", bufs=1))
        ctxn = p_cd.tile([128, ET, TOWN], BF16)
        x_own = p_cd.tile([128, ET, TOWN], F32R)
        nc.sync.dma_start(out=x_own,
                          in_=x_own_d.rearrange("(a p) s -> p a s", p=128))
        wout_sb = p_cd.tile([128, ET, ET, 128], BF16)
        nc.sync.dma_start(out=wout_sb, in_=wout_h[:, :, :, :])

        with tc.tile_pool(name="p_pr", bufs=4) as p_pr, \
             tc.tile_pool(name="p_cm", bufs=4) as p_cm, \
             tc.tile_pool(name="p_pss", bufs=3, space="PSUM") as p_pss, \
             tc.tile_pool(name="p_psx", bufs=2, space="PSUM") as p_psx:

            def block(qc, h):
                qsl = slice(qc * 512, (qc + 1) * 512)
                lo = (h % 2) * 64
                hsl = slice(lo, lo + 64)
                ot = h // 2
                ctx_ps = p_psx.tile([65, 512], F32, tag="ctx",
                                    name=f"c{qc}_{h}")
                # software pipeline: scores run 2 steps ahead of ctx so
                # the PE never waits on the scalar-engine exp.
                pr = [None] * (NT // 2)

                def scores(k2):
                    sp = p_pss.tile([128, 2, 512], F32, tag="s",
                                    name=f"s{qc}_{h}_{k2}")
                    for j in range(2):
                        kt = 2 * k2 + j
                        nc.tensor.matmul(
                            sp[:, j, :],
                            k_sb[hsl, ot, kt * 128:(kt + 1) * 128],
                            q_sb[hsl, ot, qsl], start=True, stop=True)
                    p = p_pr.tile([128, 2, 512], BF16, tag="pr")
                    nc.scalar.activation(p, sp, AF.Exp, scale=0.125)
                    pr[k2] = p

                def ctxmm(k2):
                    p = pr[k2]
                    for j in range(2):
                        kt = 2 * k2 + j
                        nc.tensor.matmul(ctx_ps, v_sb[:, kt, h, :],
                                         p[:, j, :],
                                         start=(kt == 0),
                                         stop=(kt == NT - 1))

                scores(0)
                scores(1)
                scores(2)
                for k2 in range(3, NT // 2):
                    scores(k2)
                    ctxmm(k2 - 3)
                ctxmm(NT // 2 - 3)
                ctxmm(NT // 2 - 2)
                ctxmm(NT // 2 - 1)

                # softmax denominator: recip on DVE, partition-broadcast
                # on the (otherwise idle) GpSimd engine — no PSUM needed.
                # (reciprocal_approx_fast misreads partition-offset
                # inputs, so stage the denominator at partition 0 first)
                den = p_cm.tile([1, 512], F32, tag="den")
                nc.vector.tensor_copy(den, ctx_ps[64:65, :])
                rec = p_cm.tile([1, 512], F32, tag="rec")
                nc.vector.reciprocal_approx_fast(rec, den)
                rb = p_cm.tile([64, 512], F32, tag="rbs")
                nc.gpsimd.partition_broadcast(rb, rec)
                nc.vector.tensor_tensor(ctxn[hsl, ot, qsl],
                                        ctx_ps[0:64, :], rb, OP.mult)

            for qc in range(2):
                for h in range(H):
                    block(qc, h)

        # ============ Stage D: out-proj + residual (evac on DVE) ===========
        with tc.tile_pool(name="p_dps", bufs=4, space="PSUM") as p_dps:
            for qc in range(2):
                qsl = slice(qc * 512, (qc + 1) * 512)
                for ot in range(ET):
                    ps = p_dps.tile([128, 512], F32, tag="d",
                                    name=f"d{qc}_{ot}")
                    for a in range(ET):
                        nc.tensor.matmul(ps, wout_sb[:, ot, a, :],
                                         ctxn[:, a, qsl],
                                         start=(a == 0), stop=(a == ET - 1))
                    nc.vector.scalar_tensor_tensor(
                        x2[:, ot, qsl], ps, ob_sb[:, ot:ot + 1],
                        x_own[:, ot, qsl], OP.add, OP.add)
        s_cd.close()   # ctxn/x_own/wout dead
        s_kqv.close()  # k/q/v dead

        # ============ Stage E: LN2 -> z2; Stage F: fc1+gelu -> h_sb ========
        p_z2 = ctx.enter_context(tc.tile_pool(name="p_z2", bufs=1))
        z2 = p_z2.tile([128, ET, TOWN], BF16)
        p_h = ctx.enter_context(tc.tile_pool(name="p_h", bufs=1))
        h_sb = p_h.tile([128, FT, TOWN], BF16)

        FG = 4  # fc1 weight tiles per DMA group
        with tc.tile_pool(name="p_dst", bufs=2) as p_dst, \
             tc.tile_pool(name="p_dsq", bufs=8) as p_dsq, \
             tc.tile_pool(name="p_f1w", bufs=3) as p_f1w, \
             tc.tile_pool(name="p_eps", bufs=2, space="PSUM") as p_eps, \
             tc.tile_pool(name="p_fps", bufs=4, space="PSUM") as p_fps:
            wgs = [None] * (FT // FG)

            def load_wg(g):
                wg = p_f1w.tile([128, FG, ET, 128], BF16, tag="w",
                                name=f"wf1_{g}")
                nc.sync.dma_start(out=wg, in_=wfc1_h[:, g * FG:(g + 1) * FG])
                wgs[g] = wg

            load_wg(0)  # prefetch during LN2 so fc1 never stalls
            load_wg(1)
            for qc in range(2):
                qsl = slice(qc * 512, (qc + 1) * 512)
                ps = p_eps.tile([128, 2, 512], F32, tag="e", name=f"e{qc}")
                xsqs = []
                for a in range(ET):
                    xsq = p_dsq.tile([128, 512], BF16, tag="xsq")
                    nc.vector.tensor_tensor(xsq, x2[:, a, qsl],
                                            x2[:, a, qsl], OP.mult)
                    xsqs.append(xsq)
                for a in range(ET):
                    nc.tensor.matmul(ps[:, 0, :], ones_bf, x2[:, a, qsl],
                                     start=(a == 0), stop=(a == ET - 1))
                for a in range(ET):
                    nc.tensor.matmul(ps[:, 1, :], ones_bf, xsqs[a],
                                     start=(a == 0), stop=(a == ET - 1))
                m2 = p_dst.tile([128, 512], BF16, tag="m2")
                nc.vector.tensor_scalar_mul(m2, ps[:, 0, :], inv_e)
                var = p_dst.tile([128, 512], F32, tag="var2")
                nc.vector.tensor_scalar_mul(var, ps[:, 1, :], 1.0 / (E - 1.0))
                msq = p_dst.tile([128, 512], F32, tag="msq2")
                nc.vector.tensor_tensor(msq, m2, m2, OP.mult)
                nc.vector.scalar_tensor_tensor(var, msq, -unb, var,
                                               OP.mult, OP.add)
                std = p_dst.tile([128, 512], F32, tag="std2")
                nc.scalar.activation(std, var, AF.Sqrt)
                rstd = p_dst.tile([128, 512], F32, tag="rstd2")
                nc.vector.reciprocal_approx_fast(rstd, std)
                rstd_bf = p_dst.tile([128, 512], BF16, tag="rstd2b")
                nc.vector.tensor_copy(rstd_bf, rstd)
                for a in range(ET):
                    nc.vector.tensor_tensor(z2[:, a, qsl], x2[:, a, qsl],
                                            m2, OP.subtract)
                    nc.vector.tensor_tensor(z2[:, a, qsl], z2[:, a, qsl],
                                            rstd_bf, OP.mult)

            for g in range(FT // FG):
                if g + 2 < FT // FG:
                    load_wg(g + 2)
                wg = wgs[g]
                for fl in range(FG):
                    ft = g * FG + fl
                    for qc in range(2):
                        qsl = slice(qc * 512, (qc + 1) * 512)
                        ps = p_fps.tile([128, 512], F32, tag="f",
                                        name=f"f{ft}_{qc}")
                        for a in range(ET):
                            nc.tensor.matmul(ps, wg[:, fl, a, :],
                                             z2[:, a, qsl],
                                             start=(a == 0),
                                             stop=(a == ET - 1))
                        nc.scalar.activation(h_sb[:, ft, qsl], ps, AF.Gelu,
                                             bias=f1b_sb[:, ft:ft + 1])

        # ============ Stage G: fc2 + bias + residual -> out ================
        with tc.tile_pool(name="p_f2w", bufs=2) as p_f2w, \
             tc.tile_pool(name="p_ge", bufs=4) as p_ge, \
             tc.tile_pool(name="p_gps", bufs=4, space="PSUM") as p_gps:
            for ot in range(ET):
                w2 = p_f2w.tile([128, FT, 128], BF16, tag="w",
                                name=f"wf2_{ot}")
                nc.sync.dma_start(out=w2, in_=wfc2_h[:, ot])
                for qc in range(2):
                    qsl = slice(qc * 512, (qc + 1) * 512)
                    ps = p_gps.tile([128, 512], F32, tag="g",
                                    name=f"g{ot}_{qc}")
                    for f in range(FT):
                        nc.tensor.matmul(ps, w2[:, f, :], h_sb[:, f, qsl],
                                         start=(f == 0), stop=(f == FT - 1))
                    ev = p_ge.tile([128, 512], F32, tag="ev")
                    nc.scalar.activation(ev, ps, AF.Identity,
                                         bias=f2b_sb[:, ot:ot + 1])
                    outt = p_ge.tile([128, 512], F32, tag="out")
                    nc.vector.tensor_tensor(outt, ev, x2[:, ot, qsl], OP.add)
                    nc.sync.dma_start(
                        out=out_d[ot * 128:(ot + 1) * 128, qsl], in_=outt)

    nc.finalize()
    return nc


_NC_CACHE = {}


def _get_nc():
    if "k" not in _NC_CACHE:
        _NC_CACHE["k"] = _build()
    return _NC_CACHE["k"]


def _tile_w(w_t, n_out_tiles, inner):
    # [E_in, O] (in-feature rows) -> [128, O//inner_t, E_in//128, inner] with
    # partition (e_in % 128) leading so every DMA is contiguous per partition.
    e_in, o = w_t.shape
    arr = w_t.reshape(e_in // 128, 128, n_out_tiles, o // n_out_tiles)
    return np.ascontiguousarray(arr.transpose(1, 2, 0, 3)
                                ).astype(ml_dtypes.bfloat16)


def _prepare_in_maps(inputs):
    f = np.float32
    x = np.asarray(inputs["x"], f)
    w_qkv = np.asarray(inputs["w_qkv"], np.float64)
    ln1_w = np.asarray(inputs["ln1_w"], np.float64)
    ln1_b = np.asarray(inputs["ln1_b"], np.float64)
    ln2_w = np.asarray(inputs["ln2_w"], np.float64)
    ln2_b = np.asarray(inputs["ln2_b"], np.float64)
    w_fc1 = np.asarray(inputs["w_fc1"], np.float64)

    wqkv_s = (w_qkv * ln1_w[None, :])  # fold LN1 gamma
    qkv_bias = ln1_b @ np.asarray(inputs["w_qkv"], np.float64).T  # [3E]
    wqkv_t = np.ascontiguousarray(wqkv_s.T, f)  # [E, 3E]
    wq_hh = _tile_w(wqkv_t[:, 0:E], ET, 128)
    wk_hh = _tile_w(wqkv_t[:, E:2 * E], ET, 128)
    wv_hh = _tile_w(wqkv_t[:, 2 * E:3 * E], 2, 512)
    col = lambda v: np.ascontiguousarray(
        np.asarray(v, f).reshape(-1, 128).T)  # [o] -> [128, o//128]
    qb_a = col(qkv_bias[0:E])
    kb_a = col(qkv_bias[E:2 * E])
    vb_a = np.ascontiguousarray(qkv_bias[2 * E:3 * E], f)

    wout_hh = _tile_w(np.ascontiguousarray(np.asarray(inputs["w_out"], f).T),
                      ET, 128)
    ob_a = col(inputs["b_out"])

    wfc1_s = (w_fc1 * ln2_w[None, :])
    f1b_flat = np.asarray(inputs["b_fc1"], np.float64) + ln2_b @ w_fc1.T
    f1b_a = col(f1b_flat)
    wfc1_hh = _tile_w(np.ascontiguousarray(wfc1_s.T, f), FT, 128)
    wfc2_hh = _tile_w(np.ascontiguousarray(np.asarray(inputs["w_fc2"], f).T),
                      ET, 128)
    f2b_a = col(inputs["b_fc2"])

    shared = dict(wq_h=wq_hh, wk_h=wk_hh, wv_h=wv_hh, qb=qb_a, kb=kb_a,
                  vb=vb_a, wout_h=wout_hh, ob=ob_a, wfc1_h=wfc1_hh,
                  f1b=f1b_a, wfc2_h=wfc2_hh, f2b=f2b_a)
    in_maps = []
    for core in range(NCORES):
        b, hf = divmod(core, 2)
        xs = np.roll(x[b], -hf * TOWN, axis=0)  # own tokens first; [S, E]
        x_bfc = np.ascontiguousarray(xs.T.astype(ml_dtypes.bfloat16))
        x_own = np.ascontiguousarray(xs[0:TOWN].T)  # [E, TOWN] f32
        in_maps.append(dict(x_bf=x_bfc, x_own_d=x_own, **shared))
    return in_maps


def _assemble(inputs, results):
    f = np.float32
    out = np.empty((B, S, E), f)
    for core in range(NCORES):
        b, hf = divmod(core, 2)
        out[b, hf * TOWN:(hf + 1) * TOWN, :] = results[core]["out_d"].T
    return out


def run(inputs, **spmd_kwargs):
    nc = _get_nc()
    in_maps = _prepare_in_maps(inputs)
    res = run_bass_kernel_spmd(nc, in_maps, core_ids=list(range(NCORES)),
                               **spmd_kwargs)
    return _assemble(inputs, res.results), res


def kernel(**inputs):
    out, _ = run(inputs)
    return out


# revision 38
# speedup vs baseline: 1.2827x; 1.0063x over previous
"""Encoder layer (pre-norm attention + MLP) on 8 Trainium2 cores.

Sharding: core = (batch b in 0..3, half hf in 0..1). Each core receives the
full 2048-token sequence of batch b, transposed to [E, S] and rolled so the
core's own 1024 tokens are columns 0:1024 (attention and LN are invariant to
key order, so rolling keeps the program identical across cores). The core
computes K/V over the full sequence and everything else only for its own
tokens. No collectives; the host reassembles the 8 shards.

v2 redesign vs the first version:
- Everything lives in SBUF between stages (K/Q/V, h) — no DRAM round trips.
- LN1 + QKV projection fused into one chunk-pipelined stream so the tensor
  engine stays busy (HAM stays un-throttled at 2.4 GHz).
- Attention is software-pipelined (ctx matmuls lag scores by 2 steps) so the
  PE never waits on the scalar-engine exp.
- All weights bf16 (halves HBM traffic; matmul rate identical).
- x arrives as bf16 for the LN/projection path and f32 (own tokens only)
  for the residual path.
- fc2 evacuation fuses bias + residual: single output tensor.
"""

import numpy as np
import ml_dtypes
from contextlib import ExitStack

import concourse.bacc as bacc
import concourse.mybir as mybir
import concourse.tile as tile
from concourse.bass_utils import run_bass_kernel_spmd

F32 = mybir.dt.float32
F32R = mybir.dt.float32r
BF16 = mybir.dt.bfloat16
AF = mybir.ActivationFunctionType
OP = mybir.AluOpType

B, S, E, H, D, FF = 4, 2048, 1024, 16, 64, 4096
TOWN = 1024  # tokens owned per core
ET = E // 128  # 8
FT = FF // 128  # 32
NT = S // 128  # 16 token tiles (full seq)
NCORES = 8
EPS = 1e-6
CH = 512  # token chunk for the fused LN1+QKV pipeline
NCH = S // CH  # 4
OWN_CH = TOWN // CH  # 2 (chunks 0,1 are own tokens)


def _build():
    nc = bacc.Bacc()

    x_bf = nc.dram_tensor("x_bf", [E, S], BF16, kind="ExternalInput")
    x_own_d = nc.dram_tensor("x_own_d", [E, TOWN], F32R, kind="ExternalInput")
    wq_h = nc.dram_tensor("wq_h", [128, ET, ET, 128], BF16,
                          kind="ExternalInput")
    wk_h = nc.dram_tensor("wk_h", [128, ET, ET, 128], BF16,
                          kind="ExternalInput")
    wv_h = nc.dram_tensor("wv_h", [128, 2, ET, 512], BF16,
                          kind="ExternalInput")
    qb = nc.dram_tensor("qb", [128, ET], F32, kind="ExternalInput")
    kb = nc.dram_tensor("kb", [128, ET], F32, kind="ExternalInput")
    vb = nc.dram_tensor("vb", [E], F32R, kind="ExternalInput")
    wout_h = nc.dram_tensor("wout_h", [128, ET, ET, 128], BF16,
                            kind="ExternalInput")
    ob = nc.dram_tensor("ob", [128, ET], F32, kind="ExternalInput")
    wfc1_h = nc.dram_tensor("wfc1_h", [128, FT, ET, 128], BF16,
                            kind="ExternalInput")
    f1b = nc.dram_tensor("f1b", [128, FT], F32, kind="ExternalInput")
    wfc2_h = nc.dram_tensor("wfc2_h", [128, ET, FT, 128], BF16,
                            kind="ExternalInput")
    f2b = nc.dram_tensor("f2b", [128, ET], F32, kind="ExternalInput")

    out_d = nc.dram_tensor("out_d", [E, TOWN], F32, kind="ExternalOutput")

    inv_e = 1.0 / E
    unb = float(E) / (E - 1.0)  # unbiased-variance factor

    with tile.TileContext(nc) as tc, ExitStack() as ctx:
        consts = ctx.enter_context(tc.tile_pool(name="consts", bufs=1))

        # x2 = x + attention output; bf16, lives D..G. Opened early so the
        # pool stack stays LIFO.
        p_x2 = ctx.enter_context(tc.tile_pool(name="p_x2", bufs=1))
        x2 = p_x2.tile([128, ET, TOWN], BF16)

        # ============ Stage A+B: fused LN1 + QKV over chunk pipeline =======
        s_kqv = ExitStack()
        p_kqv = s_kqv.enter_context(tc.tile_pool(name="p_kqv", bufs=1))
        k_sb = p_kqv.tile([128, ET, S], BF16)
        q_sb = p_kqv.tile([128, ET, TOWN], BF16)
        # [part = t%128, t_tile, head, 64 v dims + 1 ones col]
        v_sb = p_kqv.tile([128, NT, H, 65], BF16)

        with tc.tile_pool(name="p_w", bufs=1) as p_w, \
             tc.tile_pool(name="p_wkq", bufs=3) as p_wkq, \
             tc.tile_pool(name="p_x", bufs=2) as p_x, \
             tc.tile_pool(name="p_xsq", bufs=8) as p_xsq, \
             tc.tile_pool(name="p_z", bufs=2) as p_z, \
             tc.tile_pool(name="p_st", bufs=1) as p_st, \
             tc.tile_pool(name="p_stps", bufs=2, space="PSUM") as p_stps, \
             tc.tile_pool(name="p_pp", bufs=4, space="PSUM") as p_pp:

            xre = x_bf.rearrange("(a p) s -> p a s", p=128)
            xc_t = [None] * NCH
            z1_t = [None] * NCH

            def load_chunk(c):
                xc = p_x.tile([128, ET, CH], BF16, tag="xc", name=f"xc{c}")
                nc.sync.dma_start(out=xc, in_=xre[:, :, c * CH:(c + 1) * CH])
                xc_t[c] = xc

            # x + V weights first on the DMA ring (the critical path);
            # constants and biases behind them.
            load_chunk(0)
            wv_sb = p_w.tile([128, 2, ET, 512], BF16)
            nc.sync.dma_start(out=wv_sb, in_=wv_h[:, :, :, :])
            load_chunk(1)

            ones_f32 = consts.tile([128, 256], F32)
            nc.vector.memset(ones_f32, 1.0)
            ones_bf = consts.tile([128, 128], BF16)
            nc.vector.tensor_copy(ones_bf, ones_f32[:, 0:128])
            ones_fr = consts.tile([128, 128], F32R)
            nc.vector.tensor_copy(ones_fr, ones_f32[:, 0:128])
            ones_col = consts.tile([128, 256], BF16)
            nc.vector.tensor_copy(ones_col, ones_f32)
            nc.vector.tensor_copy(
                v_sb[:, :, :, 64],
                ones_col[:, 0:NT * H].rearrange("p (a b) -> p a b", a=NT))
            qb_sb = consts.tile([128, ET], F32)
            kb_sb = consts.tile([128, ET], F32)
            ob_sb = consts.tile([128, ET], F32)
            f1b_sb = consts.tile([128, FT], F32)
            f2b_sb = consts.tile([128, ET], F32)
            nc.sync.dma_start(out=qb_sb, in_=qb[:, :])
            nc.sync.dma_start(out=kb_sb, in_=kb[:, :])
            # v bias broadcast to all 128 token partitions (v is token-major)
            vb_row = consts.tile([1, E], F32R)
            nc.sync.dma_start(out=vb_row, in_=vb[None, :])
            vb_bc = consts.tile([128, E], F32)
            for c in range(2):
                ps = p_pp.tile([128, 512], F32, tag="pp", name=f"vbbc{c}")
                nc.tensor.matmul(ps, ones_fr[0:1, :],
                                 vb_row[:, c * 512:(c + 1) * 512],
                                 start=True, stop=True)
                nc.scalar.activation(vb_bc[:, c * 512:(c + 1) * 512], ps,
                                     AF.Copy)

            def stats(c):
                # LN1 stats for chunk c: ones-matmul sums (broadcast to all
                # 128 partitions for free), then mean/rstd in bf16.
                xc = xc_t[c]
                ps = p_stps.tile([128, 2, CH], F32, tag="st",
                                 name=f"st{c}")
                xsqs = []
                for a in range(ET):
                    xsq = p_xsq.tile([128, CH], BF16, tag="xsq")
                    nc.scalar.activation(xsq, xc[:, a, :], AF.Square)
                    xsqs.append(xsq)
                for a in range(ET):
                    nc.tensor.matmul(ps[:, 0, :], ones_bf, xc[:, a, :],
                                     start=(a == 0), stop=(a == ET - 1))
                for a in range(ET):
                    nc.tensor.matmul(ps[:, 1, :], ones_bf, xsqs[a],
                                     start=(a == 0), stop=(a == ET - 1))
                m_bf = p_st.tile([128, CH], BF16, tag="m", name=f"m{c}")
                nc.vector.tensor_scalar_mul(m_bf, ps[:, 0, :], inv_e)
                var = p_st.tile([128, CH], F32, tag="var")
                nc.vector.tensor_scalar_mul(var, ps[:, 1, :], 1.0 / (E - 1.0))
                msq = p_st.tile([128, CH], F32, tag="msq")
                nc.vector.tensor_tensor(msq, m_bf, m_bf, OP.mult)
                nc.vector.scalar_tensor_tensor(var, msq, -unb, var,
                                               OP.mult, OP.add)
                std = p_st.tile([128, CH], F32, tag="std")
                nc.scalar.activation(std, var, AF.Sqrt)
                rstd = p_st.tile([128, CH], F32, tag="rstd")
                nc.vector.reciprocal_approx_fast(rstd, std)
                rstd_bf = p_st.tile([128, CH], BF16, tag="rstdb",
                                    name=f"r{c}")
                nc.vector.tensor_copy(rstd_bf, rstd)
                # normalize
                z1 = p_z.tile([128, ET, CH], BF16, tag="z1", name=f"z1{c}")
                for a in range(ET):
                    nc.vector.tensor_tensor(z1[:, a, :], xc[:, a, :], m_bf,
                                            OP.subtract)
                    nc.vector.tensor_tensor(z1[:, a, :], z1[:, a, :], rstd_bf,
                                            OP.mult)
                z1_t[c] = z1

            def proj(c):
                z1 = z1_t[c]
                csl = slice(c * CH, (c + 1) * CH)
                # K projection (feature-major output); weights streamed
                for ot in range(ET):
                    wt = p_wkq.tile([128, ET, 128], BF16, tag="w",
                                    name=f"wk{c}_{ot}")
                    nc.sync.dma_start(out=wt, in_=wk_h[:, ot])
                    ps = p_pp.tile([128, CH], F32, tag="pp",
                                   name=f"k{c}_{ot}")
                    for a in range(ET):
                        nc.tensor.matmul(ps, wt[:, a, :], z1[:, a, :],
                                         start=(a == 0), stop=(a == ET - 1))
                    nc.scalar.activation(k_sb[:, ot, csl], ps, AF.Identity,
                                         bias=kb_sb[:, ot:ot + 1])
                # V projection (token-major output, straight into v_sb)
                for tt in range(CH // 128):
                    t_abs = c * (CH // 128) + tt
                    tsl = slice(tt * 128, (tt + 1) * 128)
                    for half in range(2):
                        ps = p_pp.tile([128, 512], F32, tag="pp",
                                       name=f"v{c}_{tt}_{half}")
                        for a in range(ET):
                            nc.tensor.matmul(ps, z1[:, a, tsl],
                                             wv_sb[:, half, a, :],
                                             start=(a == 0),
                                             stop=(a == ET - 1))
                        nc.vector.tensor_tensor(
                            v_sb[:, t_abs, half * 8:(half + 1) * 8, 0:64],
                            ps.rearrange("p (h w) -> p h w", w=64),
                            vb_bc[:, half * 512:(half + 1) * 512].rearrange(
                                "p (h w) -> p h w", w=64),
                            OP.add)
                # Q projection (own chunks only); weights streamed
                if c < OWN_CH:
                    for ot in range(ET):
                        wt = p_wkq.tile([128, ET, 128], BF16, tag="w",
                                        name=f"wq{c}_{ot}")
                        nc.sync.dma_start(out=wt, in_=wq_h[:, ot])
                        ps = p_pp.tile([128, CH], F32, tag="pp",
                                       name=f"q{c}_{ot}")
                        for a in range(ET):
                            nc.tensor.matmul(ps, wt[:, a, :], z1[:, a, :],
                                             start=(a == 0),
                                             stop=(a == ET - 1))
                        nc.scalar.activation(q_sb[:, ot, csl], ps,
                                             AF.Identity,
                                             bias=qb_sb[:, ot:ot + 1])

            # chunk pipeline: stats one chunk ahead of projections
            # x-chunk loads are emitted AFTER the preceding projection so
            # proj's streamed K/Q weight DMAs aren't queued behind 1MB
            # x transfers on the FIFO DMA ring (each xc still lands well
            # before the stats pass that consumes it).
            stats(0)
            stats(1)
            proj(0)
            # late-stage biases ride the ring behind the hot-path weights
            nc.sync.dma_start(out=ob_sb, in_=ob[:, :])
            nc.sync.dma_start(out=f1b_sb, in_=f1b[:, :])
            nc.sync.dma_start(out=f2b_sb, in_=f2b[:, :])
            load_chunk(2)
            stats(2)
            proj(1)
            load_chunk(3)
            stats(3)
            proj(2)
            proj(3)

        # ============ Stage C: attention ===================================
        # ctxn + out-proj weights + residual x, all live C..D.
        s_cd = ExitStack()
        p_cd = s_cd.enter_context(tc.tile_pool(name="p_cd", bufs=1))
        ctxn = p_cd.tile([128, ET, TOWN], BF16)
        x_own = p_cd.tile([128, ET, TOWN], F32R)
        nc.sync.dma_start(out=x_own,
                          in_=x_own_d.rearrange("(a p) s -> p a s", p=128))
        wout_sb = p_cd.tile([128, ET, ET, 128], BF16)
        nc.sync.dma_start(out=wout_sb, in_=wout_h[:, :, :, :])

        with tc.tile_pool(name="p_pr", bufs=4) as p_pr, \
             tc.tile_pool(name="p_cm", bufs=4) as p_cm, \
             tc.tile_pool(name="p_pss", bufs=3, space="PSUM") as p_pss, \
             tc.tile_pool(name="p_psx", bufs=2, space="PSUM") as p_psx:

            def block(qc, h):
                qsl = slice(qc * 512, (qc + 1) * 512)
                lo = (h % 2) * 64
                hsl = slice(lo, lo + 64)
                ot = h // 2
                ctx_ps = p_psx.tile([65, 512], F32, tag="ctx",
                                    name=f"c{qc}_{h}")
                # software pipeline: scores run 2 steps ahead of ctx so
                # the PE never waits on the scalar-engine exp.
                pr = [None] * (NT // 2)

                def scores(k2):
                    sp = p_pss.tile([128, 2, 512], F32, tag="s",
                                    name=f"s{qc}_{h}_{k2}")
                    for j in range(2):
                        kt = 2 * k2 + j
                        nc.tensor.matmul(
                            sp[:, j, :],
                            k_sb[hsl, ot, kt * 128:(kt + 1) * 128],
                            q_sb[hsl, ot, qsl], start=True, stop=True)
                    p = p_pr.tile([128, 2, 512], BF16, tag="pr")
                    nc.scalar.activation(p, sp, AF.Exp, scale=0.125)
                    pr[k2] = p

                def ctxmm(k2):
                    p = pr[k2]
                    for j in range(2):
                        kt = 2 * k2 + j
                        nc.tensor.matmul(ctx_ps, v_sb[:, kt, h, :],
                                         p[:, j, :],
                                         start=(kt == 0),
                                         stop=(kt == NT - 1))

                scores(0)
                scores(1)
                scores(2)
                for k2 in range(3, NT // 2):
                    scores(k2)
                    ctxmm(k2 - 3)
                ctxmm(NT // 2 - 3)
                ctxmm(NT // 2 - 2)
                ctxmm(NT // 2 - 1)

                # softmax denominator: recip on DVE, partition-broadcast
                # on the (otherwise idle) GpSimd engine — no PSUM needed.
                # (reciprocal_approx_fast misreads partition-offset
                # inputs, so stage the denominator at partition 0 first)
                den = p_cm.tile([1, 512], F32, tag="den")
                nc.vector.tensor_copy(den, ctx_ps[64:65, :])
                rec = p_cm.tile([1, 512], F32, tag="rec")
                nc.vector.reciprocal_approx_fast(rec, den)
                rb = p_cm.tile([64, 512], F32, tag="rbs")
                nc.gpsimd.partition_broadcast(rb, rec)
                nc.vector.tensor_tensor(ctxn[hsl, ot, qsl],
                                        ctx_ps[0:64, :], rb, OP.mult)

            for qc in range(2):
                for h in range(H):
                    block(qc, h)

        # ============ Stage D: out-proj + residual (evac on DVE) ===========
        with tc.tile_pool(name="p_dps", bufs=4, space="PSUM") as p_dps:
            for qc in range(2):
                qsl = slice(qc * 512, (qc + 1) * 512)
                for ot in range(ET):
                    ps = p_dps.tile([128, 512], F32, tag="d",
                                    name=f"d{qc}_{ot}")
                    for a in range(ET):
                        nc.tensor.matmul(ps, wout_sb[:, ot, a, :],
                                         ctxn[:, a, qsl],
                                         start=(a == 0), stop=(a == ET - 1))
                    nc.vector.scalar_tensor_tensor(
                        x2[:, ot, qsl], ps, ob_sb[:, ot:ot + 1],
                        x_own[:, ot, qsl], OP.add, OP.add)
        s_cd.close()   # ctxn/x_own/wout dead
        s_kqv.close()  # k/q/v dead

        # ============ Stage E: LN2 -> z2; Stage F: fc1+gelu -> h_sb ========
        p_z2 = ctx.enter_context(tc.tile_pool(name="p_z2", bufs=1))
        z2 = p_z2.tile([128, ET, TOWN], BF16)
        p_h = ctx.enter_context(tc.tile_pool(name="p_h", bufs=1))
        h_sb = p_h.tile([128, FT, TOWN], BF16)

        FG = 4  # fc1 weight tiles per DMA group
        with tc.tile_pool(name="p_dst", bufs=2) as p_dst, \
             tc.tile_pool(name="p_dsq", bufs=8) as p_dsq, \
             tc.tile_pool(name="p_f1w", bufs=3) as p_f1w, \
             tc.tile_pool(name="p_eps", bufs=2, space="PSUM") as p_eps, \
             tc.tile_pool(name="p_fps", bufs=4, space="PSUM") as p_fps:
            wgs = [None] * (FT // FG)

            def load_wg(g):
                wg = p_f1w.tile([128, FG, ET, 128], BF16, tag="w",
                                name=f"wf1_{g}")
                nc.sync.dma_start(out=wg, in_=wfc1_h[:, g * FG:(g + 1) * FG])
                wgs[g] = wg

            load_wg(0)  # prefetch during LN2 so fc1 never stalls
            load_wg(1)
            for qc in range(2):
                qsl = slice(qc * 512, (qc + 1) * 512)
                ps = p_eps.tile([128, 2, 512], F32, tag="e", name=f"e{qc}")
                xsqs = []
                for a in range(ET):
                    xsq = p_dsq.tile([128, 512], BF16, tag="xsq")
                    nc.vector.tensor_tensor(xsq, x2[:, a, qsl],
                                            x2[:, a, qsl], OP.mult)
                    xsqs.append(xsq)
                for a in range(ET):
                    nc.tensor.matmul(ps[:, 0, :], ones_bf, x2[:, a, qsl],
                                     start=(a == 0), stop=(a == ET - 1))
                for a in range(ET):
                    nc.tensor.matmul(ps[:, 1, :], ones_bf, xsqs[a],
                                     start=(a == 0), stop=(a == ET - 1))
                m2 = p_dst.tile([128, 512], BF16, tag="m2")
                nc.vector.tensor_scalar_mul(m2, ps[:, 0, :], inv_e)
                var = p_dst.tile([128, 512], F32, tag="var2")
                nc.vector.tensor_scalar_mul(var, ps[:, 1, :], 1.0 / (E - 1.0))
                msq = p_dst.tile([128, 512], F32, tag="msq2")
                nc.vector.tensor_tensor(msq, m2, m2, OP.mult)
                nc.vector.scalar_tensor_tensor(var, msq, -unb, var,
                                               OP.mult, OP.add)
                std = p_dst.tile([128, 512], F32, tag="std2")
                nc.scalar.activation(std, var, AF.Sqrt)
                rstd = p_dst.tile([128, 512], F32, tag="rstd2")
                nc.vector.reciprocal_approx_fast(rstd, std)
                rstd_bf = p_dst.tile([128, 512], BF16, tag="rstd2b")
                nc.vector.tensor_copy(rstd_bf, rstd)
                for a in range(ET):
                    nc.vector.tensor_tensor(z2[:, a, qsl], x2[:, a, qsl],
                                            m2, OP.subtract)
                    nc.vector.tensor_tensor(z2[:, a, qsl], z2[:, a, qsl],
                                            rstd_bf, OP.mult)

            for g in range(FT // FG):
                if g + 2 < FT // FG:
                    load_wg(g + 2)
                wg = wgs[g]
                for fl in range(FG):
                    ft = g * FG + fl
                    for qc in range(2):
                        qsl = slice(qc * 512, (qc + 1) * 512)
                        ps = p_fps.tile([128, 512], F32, tag="f",
                                        name=f"f{ft}_{qc}")
                        for a in range(ET):
                            nc.tensor.matmul(ps, wg[:, fl, a, :],
                                             z2[:, a, qsl],
                                             start=(a == 0),
                                             stop=(a == ET - 1))
                        nc.scalar.activation(h_sb[:, ft, qsl], ps, AF.Gelu,
                                             bias=f1b_sb[:, ft:ft + 1])

        # ============ Stage G: fc2 + bias + residual -> out ================
        with tc.tile_pool(name="p_f2w", bufs=2) as p_f2w, \
             tc.tile_pool(name="p_ge", bufs=4) as p_ge, \
             tc.tile_pool(name="p_gps", bufs=4, space="PSUM") as p_gps:
            for ot in range(ET):
                w2 = p_f2w.tile([128, FT, 128], BF16, tag="w",
                                name=f"wf2_{ot}")
                nc.sync.dma_start(out=w2, in_=wfc2_h[:, ot])
                for qc in range(2):
                    qsl = slice(qc * 512, (qc + 1) * 512)
                    ps = p_gps.tile([128, 512], F32, tag="g",
                                    name=f"g{ot}_{qc}")
                    for f in range(FT):
                        nc.tensor.matmul(ps, w2[:, f, :], h_sb[:, f, qsl],
                                         start=(f == 0), stop=(f == FT - 1))
                    ev = p_ge.tile([128, 512], F32, tag="ev")
                    nc.scalar.activation(ev, ps, AF.Identity,
                                         bias=f2b_sb[:, ot:ot + 1])
                    outt = p_ge.tile([128, 512], F32, tag="out")
                    nc.vector.tensor_tensor(outt, ev, x2[:, ot, qsl], OP.add)
                    nc.sync.dma_start(
                        out=out_d[ot * 128:(ot + 1) * 128, qsl], in_=outt)

    nc.finalize()
    return nc


_NC_CACHE = {}


def _get_nc():
    if "k" not in _NC_CACHE:
        _NC_CACHE["k"] = _build()
    return _NC_CACHE["k"]


def _tile_w(w_t, n_out_tiles, inner):
    # [E_in, O] (in-feature rows) -> [128, O//inner_t, E_in//128, inner] with
    # partition (e_in % 128) leading so every DMA is contiguous per partition.
    e_in, o = w_t.shape
    arr = w_t.reshape(e_in // 128, 128, n_out_tiles, o // n_out_tiles)
    return np.ascontiguousarray(arr.transpose(1, 2, 0, 3)
                                ).astype(ml_dtypes.bfloat16)


def _prepare_in_maps(inputs):
    f = np.float32
    x = np.asarray(inputs["x"], f)
    w_qkv = np.asarray(inputs["w_qkv"], np.float64)
    ln1_w = np.asarray(inputs["ln1_w"], np.float64)
    ln1_b = np.asarray(inputs["ln1_b"], np.float64)
    ln2_w = np.asarray(inputs["ln2_w"], np.float64)
    ln2_b = np.asarray(inputs["ln2_b"], np.float64)
    w_fc1 = np.asarray(inputs["w_fc1"], np.float64)

    wqkv_s = (w_qkv * ln1_w[None, :])  # fold LN1 gamma
    qkv_bias = ln1_b @ np.asarray(inputs["w_qkv"], np.float64).T  # [3E]
    wqkv_t = np.ascontiguousarray(wqkv_s.T, f)  # [E, 3E]
    wq_hh = _tile_w(wqkv_t[:, 0:E], ET, 128)
    wk_hh = _tile_w(wqkv_t[:, E:2 * E], ET, 128)
    wv_hh = _tile_w(wqkv_t[:, 2 * E:3 * E], 2, 512)
    col = lambda v: np.ascontiguousarray(
        np.asarray(v, f).reshape(-1, 128).T)  # [o] -> [128, o//128]
    qb_a = col(qkv_bias[0:E])
    kb_a = col(qkv_bias[E:2 * E])
    vb_a = np.ascontiguousarray(qkv_bias[2 * E:3 * E], f)

    wout_hh = _tile_w(np.ascontiguousarray(np.asarray(inputs["w_out"], f).T),
                      ET, 128)
    ob_a = col(inputs["b_out"])

    wfc1_s = (w_fc1 * ln2_w[None, :])
    f1b_flat = np.asarray(inputs["b_fc1"], np.float64) + ln2_b @ w_fc1.T
    f1b_a = col(f1b_flat)
    wfc1_hh = _tile_w(np.ascontiguousarray(wfc1_s.T, f), FT, 128)
    wfc2_hh = _tile_w(np.ascontiguousarray(np.asarray(inputs["w_fc2"], f).T),
                      ET, 128)
    f2b_a = col(inputs["b_fc2"])

    shared = dict(wq_h=wq_hh, wk_h=wk_hh, wv_h=wv_hh, qb=qb_a, kb=kb_a,
                  vb=vb_a, wout_h=wout_hh, ob=ob_a, wfc1_h=wfc1_hh,
                  f1b=f1b_a, wfc2_h=wfc2_hh, f2b=f2b_a)
    in_maps = []
    for core in range(NCORES):
        b, hf = divmod(core, 2)
        xs = np.roll(x[b], -hf * TOWN, axis=0)  # own tokens first; [S, E]
        x_bfc = np.ascontiguousarray(xs.T.astype(ml_dtypes.bfloat16))
        x_own = np.ascontiguousarray(xs[0:TOWN].T)  # [E, TOWN] f32
        in_maps.append(dict(x_bf=x_bfc, x_own_d=x_own, **shared))
    return in_maps


def _assemble(inputs, results):
    f = np.float32
    out = np.empty((B, S, E), f)
    for core in range(NCORES):
        b, hf = divmod(core, 2)
        out[b, hf * TOWN:(hf + 1) * TOWN, :] = results[core]["out_d"].T
    return out


def run(inputs, **spmd_kwargs):
    nc = _get_nc()
    in_maps = _prepare_in_maps(inputs)
    res = run_bass_kernel_spmd(nc, in_maps, core_ids=list(range(NCORES)),
                               **spmd_kwargs)
    return _assemble(inputs, res.results), res


def kernel(**inputs):
    out, _ = run(inputs)
    return out
